# revision 38
# baseline (speedup 1.0000x reference)
"""LundNetTagger GNN on 8 Trainium2 NeuronCores (Bass/Tile).

Self-contained: kernel(**inputs) -> np.ndarray [1000, 2] float32.

Strategy: nodes are assigned to 100352 "slots" (8 cores x 98 windows x 128),
packed so each window receives <= 512 edges. Edges live on the core owning
their dst slot, in window-major order padded to 4x128-edge chunks per window.
Per-edge MLPs run in bf16 feature-major layout; EdgeConv cat[xi, xj-xi] is
folded into split weights WA = W[:C]-W[C:], WB = W[C:]. GraphNorm stats are
global AllGathers of per-core sums. Mean-aggregation is a collision-free
one-hot matmul scatter into PSUM per window. Node tables are AllGathered in
bf16 (Shared outputs) between convs; src-side gathers use indirect DMA.
Intermediate z activations live in a persistent SBUF buffer (mb0) with only
the mb1 half round-tripping DRAM for the 256-wide convs.
"""
import numpy as np
import ml_dtypes

import concourse.bass as bass
import concourse.tile as tile
from concourse import bacc, mybir
from concourse.bass_utils import run_bass_kernel_spmd

BF16 = mybir.dt.bfloat16
F32 = mybir.dt.float32
AOP = mybir.AluOpType
AFT = mybir.ActivationFunctionType
AX = mybir.AxisListType

N_NODES = 100000
N_EDGES = 400000
N_GRAPHS = 1000
NC = 8
WIN = 128
NWIN = 98
SPC = WIN * NWIN          # 12544
NSLOTS = SPC * NC         # 100352
QUAD = NSLOTS // 4        # 25088
B = 4                     # chunks per window
EPW = B * WIN             # 512
E_PAD = NWIN * EPW        # 50176
EPS = 1e-5

NW_BLK = 7
BLK = NW_BLK * EPW        # 3584
NBLK = NWIN // NW_BLK     # 14
NCHUNK = BLK // 128       # 28
NSEG = BLK // 512         # 7


_cache = {}


# ============================ host-side packing ============================

def _pack(edge_index, batch):
    src = np.asarray(edge_index[0], dtype=np.int64)
    dst = np.asarray(edge_index[1], dtype=np.int64)
    batch = np.asarray(batch, dtype=np.int64)
    cnt = np.bincount(dst, minlength=N_NODES)

    nvirt = NSLOTS - N_NODES
    cnt_all = np.concatenate([cnt, np.zeros(nvirt, dtype=cnt.dtype)])
    order = np.argsort(-cnt_all, kind="stable")
    GW = NWIN * NC
    rounds = NSLOTS // GW
    win_of_rank = np.empty(NSLOTS, dtype=np.int64)
    for r in range(rounds):
        seg = np.arange(GW) if r % 2 == 0 else np.arange(GW - 1, -1, -1)
        win_of_rank[r * GW:(r + 1) * GW] = seg
    win_of_node = np.empty(NSLOTS, dtype=np.int64)
    win_of_node[order] = win_of_rank
    wsum = np.bincount(win_of_node, weights=cnt_all.astype(np.float64),
                       minlength=GW).astype(np.int64)

    cap = EPW
    members_of = [list(np.where(win_of_node == w)[0]) for w in range(GW)]
    for _ in range(2000):
        over = np.where(wsum > cap)[0]
        if len(over) == 0:
            break
        w = int(over[0])
        # smallest-count >0 node in w
        mem = members_of[w]
        cs = [(int(cnt_all[n]), n) for n in mem if cnt_all[n] > 0]
        cs.sort()
        moved = False
        for c1, n in cs:
            # find target window with a smaller-count node to swap
            worder2 = np.argsort(wsum)
            for tw in worder2[:64]:
                tw = int(tw)
                if tw == w:
                    continue
                tmem = members_of[tw]
                best = None
                for m in tmem:
                    c2 = int(cnt_all[m])
                    if c2 < c1 and wsum[tw] + c1 - c2 <= cap:
                        if best is None or c2 < best[0]:
                            best = (c2, m)
                        if c2 == 0:
                            break
                if best is not None:
                    c2, m = best
                    members_of[tw].remove(m)
                    members_of[tw].append(n)
                    members_of[w].remove(n)
                    members_of[w].append(m)
                    win_of_node[n] = tw
                    win_of_node[m] = w
                    wsum[tw] += c1 - c2
                    wsum[w] -= c1 - c2
                    moved = True
                    break
            if moved:
                break
        if not moved:
            raise RuntimeError("packing fixup stuck")
    assert wsum.max() <= cap, f"window packing failed: max={wsum.max()}"

    worder = np.argsort(-wsum, kind="stable")
    core_load = np.zeros(NC, dtype=np.int64)
    core_nwin = np.zeros(NC, dtype=np.int64)
    core_of_win = np.empty(GW, dtype=np.int64)
    for w in worder:
        cands = np.where(core_nwin < NWIN)[0]
        c = cands[np.argmin(core_load[cands])]
        core_of_win[w] = c
        core_load[c] += wsum[w]
        core_nwin[c] += 1

    win_lists = [[] for _ in range(NC)]
    for w in range(GW):
        win_lists[core_of_win[w]].append(w)
    for c in range(NC):
        wl = win_lists[c]
        j = int(np.argmin(wsum[wl]))
        assert wsum[wl[j]] < cap, "no sentinel room"
        wl[j], wl[-1] = wl[-1], wl[j]

    slot_of_node = np.empty(NSLOTS, dtype=np.int64)
    for c in range(NC):
        for wi, w in enumerate(win_lists[c]):
            mem = np.sort(np.array(members_of[w], dtype=np.int64))
            assert len(mem) == WIN
            slot_of_node[mem] = c * SPC + wi * WIN + np.arange(WIN)
    node_of_slot = np.empty(NSLOTS, dtype=np.int64)
    node_of_slot[slot_of_node] = np.arange(NSLOTS)
    cnt_of_slot = cnt_all[node_of_slot]

    qzero = []
    for q in range(4):
        z = np.where(cnt_of_slot[q * QUAD:(q + 1) * QUAD] == 0)[0]
        assert len(z) > 0
        qzero.append(int(z[0]))  # local to quadrant
    czero = []
    for c in range(NC):
        z = np.where(cnt_of_slot[c * SPC:(c + 1) * SPC] == 0)[0]
        assert len(z) > 0
        czero.append(int(z[0]))  # local to core

    dslot = slot_of_node[dst]
    sslot = slot_of_node[src]
    ecore = dslot // SPC
    ewin = (dslot % SPC) // WIN
    key = ecore * (NWIN * WIN) + ewin * WIN + (dslot % WIN)
    eorder = np.argsort(key, kind="stable")
    dsl, ssl = dslot[eorder], sslot[eorder]
    ec, ew = ecore[eorder], ewin[eorder]

    cw = ec * NWIN + ew
    cw_cnt = np.bincount(cw, minlength=NC * NWIN)
    assert cw_cnt.max() <= EPW

    xi_idx = np.zeros((NC, E_PAD), dtype=np.int64)
    xj_idx = np.zeros((NC, E_PAD), dtype=np.int64)
    dstwin = np.full((NC, E_PAD), -1.0, dtype=np.float32)
    valid = np.zeros((NC, E_PAD), dtype=bool)

    ofs = (np.arange(NC * NWIN) % NWIN) * EPW
    start = np.concatenate([[0], np.cumsum(cw_cnt)[:-1]])
    within = np.arange(N_EDGES) - start[cw]
    pos = ofs[cw] + within
    xi_idx[ec, pos] = dsl % SPC
    xj_idx[ec, pos] = ssl
    dstwin[ec, pos] = (dsl % WIN).astype(np.float32)
    valid[ec, pos] = True
    for c in range(NC):
        xi_idx[c, ~valid[c]] = czero[c]
    pad_cnt = (~valid).sum(axis=1).astype(np.float32)
    assert np.all(~valid[:, -1]), "sentinel column must be padding"

    gzero = qzero[0]  # global slot with zero row
    xj_glob = np.where(valid, xj_idx, gzero).astype(np.int32)

    inv_cnt = (1.0 / np.maximum(cnt_of_slot.reshape(NC, SPC), 1.0)).astype(np.float32)

    g_of_slot = np.full(NSLOTS, -1, dtype=np.int64)
    real = node_of_slot < N_NODES
    g_of_slot[real] = batch[node_of_slot[real]]
    NGW = 8
    Bg = 0
    pools = [[None] * NGW for _ in range(NC)]
    for c in range(NC):
        gl = g_of_slot[c * SPC:(c + 1) * SPC]
        for gw in range(NGW):
            m = np.where((gl >= gw * 128) & (gl < (gw + 1) * 128))[0]
            pools[c][gw] = m
            Bg = max(Bg, (len(m) + 127) // 128)
    NPG = Bg * 128
    pool_idx = np.zeros((NC, NGW, NPG), dtype=np.int16)
    pool_gwl = np.full((NC, NGW, NPG), -1.0, dtype=np.float32)
    for c in range(NC):
        for gw in range(NGW):
            m = pools[c][gw]
            pool_idx[c, gw, :len(m)] = m.astype(np.int16)
            pool_idx[c, gw, len(m):] = czero[c]
            pool_gwl[c, gw, :len(m)] = (g_of_slot[c * SPC + m] - gw * 128).astype(np.float32)

    gcnt = np.bincount(batch, minlength=N_GRAPHS).astype(np.float32)
    inv_g = np.zeros(1024, dtype=np.float32)
    inv_g[:N_GRAPHS] = 1.0 / np.maximum(gcnt, 1.0)

    return dict(slot_of_node=slot_of_node, node_of_slot=node_of_slot,
                xj_glob=xj_glob, dstwin=dstwin, pad_cnt=pad_cnt,
                inv_cnt=inv_cnt, valid=valid, eorder=eorder, ec=ec, pos=pos,
                pool_idx=pool_idx, pool_gwl=pool_gwl, inv_g=inv_g, Bg=Bg)


def _bf(x):
    return np.ascontiguousarray(np.asarray(x, dtype=np.float32)).astype(ml_dtypes.bfloat16)


def _tile_w(w):
    K, M = w.shape
    nk, nm = (K + 127) // 128, (M + 127) // 128
    out = np.zeros((nk, nm, 128, 128), dtype=ml_dtypes.bfloat16)
    for i in range(nk):
        for j in range(nm):
            blk = np.asarray(w, dtype=np.float32)[i * 128:(i + 1) * 128, j * 128:(j + 1) * 128]
            out[i, j, :blk.shape[0], :blk.shape[1]] = _bf(blk)
    return out


# ============================ device kernel ============================

EQ = E_PAD // 4           # 12544 edges per msgT quarter
EQP = 12800               # quarter padded to 25x512
NSEGQ = EQP // 512        # 25


def _build(Bg, debug=False, phases=4):
    nc = bacc.Bacc("TRN2", target_bir_lowering=False, debug=False, num_devices=NC)

    def din(name, shape, dt):
        return nc.dram_tensor(name, shape, dt, kind="ExternalInput").ap()

    t_msgT = din("msgT", [128, EQP], BF16)
    t_xj = din("xj_idx", [128, E_PAD // 128], mybir.dt.int32)
    t_dstwin = din("dstwin", [128, E_PAD // 128], F32)
    t_dwrow = din("dwrow", [1, E_PAD], BF16)
    t_invcnt = din("invcnt", [128, NWIN], F32)
    t_padcnt = din("padcnt", [128, 1], F32)
    t_iota = din("iota", [128, 128], F32)
    t_iotap = din("iotap", [128, 1], F32)
    t_ident = din("ident", [128, 128], BF16)
    t_c1w = din("c1w", [3, 128, 128], BF16)
    t_c1b = din("c1b", [3, 128, 1], F32)
    t_c1gn = din("c1gn", [3, 3, 128, 1], F32)
    t_c2wa = din("c2wa", [2, 128, 128], BF16)
    t_c2wb = din("c2wb", [2, 128, 128], BF16)
    t_c2w2 = din("c2w2", [2, 2, 128, 128], BF16)
    t_c2b = din("c2b", [2, 2, 128, 1], F32)
    t_c2gn = din("c2gn", [2, 3, 2, 128, 1], F32)
    t_c3wa = din("c3wa", [2, 2, 128, 128], BF16)
    t_c3wbv = din("c3wbv", [2, 128, 256], BF16)
    t_c3b = din("c3b", [2, 128, 1], F32)
    t_c3gn = din("c3gn", [3, 2, 128, 1], F32)
    t_lw1 = din("lw1", [2, 2, 128, 128], BF16)
    t_lb1 = din("lb1", [2, 128, 1], F32)
    t_lw2 = din("lw2", [2, 128, 2], BF16)
    t_lb2 = din("lb2", [2, 1], F32)
    t_pidx = din("pool_idx", [8, 128, Bg], mybir.dt.int32)
    t_pgwl = din("pool_gwl", [128, 8 * Bg], F32)
    t_invg = din("invg", [128, 1], F32)

    o_out = nc.dram_tensor("out", [2, 128], F32, kind="ExternalOutput").ap()
    dbg = {}
    if debug:
        dbg["x1"] = nc.dram_tensor("dbg_x1", [NSLOTS, 128], BF16, kind="ExternalOutput").ap()
        dbg["x2"] = nc.dram_tensor("dbg_x2", [NSLOTS, 256], BF16, kind="ExternalOutput").ap()
        dbg["x3"] = nc.dram_tensor("dbg_x3", [SPC, 256], BF16, kind="ExternalOutput").ap()
        dbg["pool"] = nc.dram_tensor("dbg_pool", [128, 256], F32, kind="ExternalOutput").ap()

    with tile.TileContext(nc) as tc:
        with tc.tile_pool(name="dram", bufs=1, space="DRAM") as dram, \
             tc.tile_pool(name="cp", bufs=1) as cp, \
             tc.tile_pool(name="zp0", bufs=1) as zpool:
            # persistent SBUF z buffer (mb0 half of every conv's activations)
            zs = zpool.tile([128, E_PAD], BF16, name="zs")            # 12.8MB

            z_scr = [dram.tile([128, E_PAD], BF16, tag=f"zscr{i}", name=f"zscr{i}")
                     for i in range(2)]
            tab1_loc = dram.tile([SPC, 128], BF16)
            tab1 = dram.tile([NSLOTS, 128], BF16, addr_space="Shared")
            tab2_loc = dram.tile([SPC, 256], BF16)
            vt_loc = dram.tile([SPC, 256], BF16)
            vtab = dram.tile([NSLOTS, 256], BF16, addr_space="Shared")
            if debug:
                tab2 = dram.tile([NSLOTS, 256], BF16, addr_space="Shared")
            tab3_loc = dram.tile([SPC, 256], BF16)
            st_in = dram.tile([128, 8], F32)
            _agn = [0]
            pool_in = dram.tile([1024, 256], F32)
            pool_rs = dram.tile([128, 256], F32)

            ident = cp.tile([128, 128], BF16)
            nc.sync.dma_start(ident[:], t_ident[:])
            iota = cp.tile([128, 128], F32)
            nc.sync.dma_start(iota[:], t_iota[:])
            iotap = cp.tile([128, 1], F32)
            nc.sync.dma_start(iotap[:], t_iotap[:])
            invcnt = cp.tile([128, NWIN], F32)
            nc.sync.dma_start(invcnt[:], t_invcnt[:])
            dwin = cp.tile([128, E_PAD // 128], F32)
            nc.sync.dma_start(dwin[:], t_dstwin[:])
            padcnt = cp.tile([128, 1], F32)
            nc.sync.dma_start(padcnt[:], t_padcnt[:])
            zerocol = cp.tile([128, 1], F32)
            nc.vector.memset(zerocol[:], 0.0)
            iotab = cp.tile([128, 128], BF16)
            nc.vector.tensor_copy(iotab[:], iota[:])
            dwinb = cp.tile([128, E_PAD // 128], BF16)
            nc.vector.tensor_copy(dwinb[:], dwin[:])

            # ---------- helpers ----------
            def allgather_stats(s_acc, q_acc, n_mb, sb):
                # Shared DRAM allows a single writer inst: fresh tile per call
                _agn[0] += 1
                st_ag = dram.tile([128 * NC, 8], F32, addr_space="Shared",
                                  tag=f"st_ag{_agn[0]}", name=f"st_ag{_agn[0]}")
                st = sb.tile([128, 8], F32, tag="st_")
                nc.vector.memset(st[:], 0.0)
                nc.vector.tensor_copy(st[:, 0:n_mb], s_acc[:])
                nc.vector.tensor_copy(st[:, 4:4 + n_mb], q_acc[:])
                nc.sync.dma_start(st_in[:], st[:])
                nc.gpsimd.collective_compute(
                    "AllGather", AOP.bypass, replica_groups=[list(range(NC))],
                    ins=[st_in.opt()], outs=[st_ag.opt()])
                stg8 = sb.tile([128, 8, 8], F32, tag="stg8_")
                nc.sync.dma_start(stg8[:],
                                  st_ag[:].rearrange("(g p) j -> p g j", g=NC))
                stg = sb.tile([128, 8], F32, tag="stg_")
                nc.vector.tensor_reduce(
                    out=stg[:], in_=stg8[:].rearrange("p g j -> p j g"),
                    axis=AX.X, op=AOP.add)
                return stg

            def affine_from_stats(stg, n_mb, b_lin, gn, sb):
                A, Cc = [], []
                for mb in range(n_mb):
                    s = stg[:, mb:mb + 1]
                    q = stg[:, 4 + mb:5 + mb]
                    g, bgn, ms = gn[0][mb], gn[1][mb], gn[2][mb]
                    bl = b_lin[mb]
                    m = sb.tile([128, 1], F32, tag="af_m")
                    nc.vector.tensor_scalar(m[:], s, 1.0 / N_EDGES, None, AOP.mult)
                    nc.vector.tensor_tensor(m[:], m[:], bl, op=AOP.add)
                    e2 = sb.tile([128, 1], F32, tag="af_e2")
                    nc.vector.tensor_scalar(e2[:], q, 1.0 / N_EDGES, None, AOP.mult)
                    tmp = sb.tile([128, 1], F32, tag="af_t")
                    nc.vector.tensor_tensor(tmp[:], m[:], bl, op=AOP.mult)
                    nc.vector.tensor_scalar(tmp[:], tmp[:], 2.0, None, AOP.mult)
                    nc.vector.tensor_tensor(e2[:], e2[:], tmp[:], op=AOP.add)
                    nc.vector.tensor_tensor(tmp[:], bl, bl, op=AOP.mult)
                    nc.vector.tensor_tensor(e2[:], e2[:], tmp[:], op=AOP.subtract)
                    msm = sb.tile([128, 1], F32, tag="af_msm")
                    nc.vector.tensor_tensor(msm[:], ms, m[:], op=AOP.mult)
                    var = sb.tile([128, 1], F32, tag="af_v")
                    nc.vector.tensor_tensor(var[:], msm[:], msm[:], op=AOP.mult)
                    nc.vector.tensor_tensor(tmp[:], msm[:], m[:], op=AOP.mult)
                    nc.vector.tensor_scalar(tmp[:], tmp[:], 2.0, None, AOP.mult)
                    nc.vector.tensor_tensor(var[:], var[:], tmp[:], op=AOP.subtract)
                    nc.vector.tensor_tensor(var[:], var[:], e2[:], op=AOP.add)
                    a = sb.tile([128, 1], F32, tag="af_a")
                    nc.vector.tensor_scalar(var[:], var[:], EPS, None, AOP.add)
                    nc.scalar.activation(a[:], var[:], AFT.Sqrt)
                    nc.vector.reciprocal(a[:], a[:])
                    nc.vector.tensor_tensor(a[:], a[:], g, op=AOP.mult)
                    cc = sb.tile([128, 1], F32, tag="af_c")
                    nc.vector.tensor_tensor(cc[:], bl, msm[:], op=AOP.subtract)
                    nc.vector.tensor_tensor(cc[:], cc[:], a[:], op=AOP.mult)
                    nc.vector.tensor_tensor(cc[:], cc[:], bgn, op=AOP.add)
                    A.append(a)
                    Cc.append(cc)
                return A, Cc

            def zsink(zp_ap, dst_ap, s_col, sb, par=0):
                """PSUM -> bf16 dst copy fused with column-sum accumulation,
                s_col += colsum; alternates ACT / DVE by parity."""
                sa = sb.tile([128, 1], F32, tag="zk_sa")
                if par % 2 == 0:
                    nc.scalar.activation(dst_ap, zp_ap, AFT.Copy, accum_out=sa[:])
                else:
                    nc.vector.tensor_copy(dst_ap, zp_ap)
                    nc.vector.reduce_sum(out=sa[:], in_=zp_ap, axis=AX.X)
                nc.vector.tensor_tensor(s_col, s_col, sa[:], op=AOP.add)

            def blk_sq(src_ap, q_col, sb, par=0):
                """q_col += column sum-of-squares of bf16 [128, n] block;
                alternates ACT / DVE by parity."""
                n = src_ap.shape[-1]
                sq = sb.tile([128, BLK], BF16, tag="bs_sq")
                qa = sb.tile([128, 1], F32, tag="bs_qa")
                if par % 2 == 0:
                    nc.scalar.activation(sq[:, :n], src_ap, AFT.Square,
                                         accum_out=qa[:])
                else:
                    nc.vector.tensor_tensor(sq[:, :n], src_ap, src_ap,
                                            op=AOP.mult)
                    nc.vector.reduce_sum(out=qa[:], in_=sq[:, :n], axis=AX.X)
                nc.vector.tensor_tensor(q_col, q_col, qa[:], op=AOP.add)

            def sentinel_correct(s_acc, q_acc, zsent_cols, n_mb, sb):
                for mb in range(n_mb):
                    zs_ = zsent_cols[mb]
                    t1 = sb.tile([128, 1], F32, tag="sc_t1")
                    nc.vector.tensor_tensor(t1[:], zs_, padcnt[:], op=AOP.mult)
                    nc.vector.tensor_tensor(s_acc[:, mb:mb + 1], s_acc[:, mb:mb + 1],
                                            t1[:], op=AOP.subtract)
                    nc.vector.tensor_tensor(t1[:], zs_, zs_, op=AOP.mult)
                    nc.vector.tensor_tensor(t1[:], t1[:], padcnt[:], op=AOP.mult)
                    nc.vector.tensor_tensor(q_acc[:, mb:mb + 1], q_acc[:, mb:mb + 1],
                                            t1[:], op=AOP.subtract)

            def load_vec(t_ap, sb, tag):
                v = sb.tile([128, 1], F32, tag=tag)
                nc.sync.dma_start(v[:], t_ap)
                return v[:]

            def scatter_pass(zdram, n_mb, A, Cc, tab_loc, Cout, vw=None, vdst=None):
                """h = relu(A z + C) per mb; mb0 z from zs SBUF, mb1 from zdram.
                Mean-scatter into tab_loc DRAM; optionally also emit
                V = tab @ Wb rows into vdst."""
                with tc.tile_pool(name="sc_sb", bufs=2) as sb, \
                     tc.tile_pool(name="sc_tp", bufs=2, space="PSUM") as ps_tp, \
                     tc.tile_pool(name="sc_v", bufs=2, space="PSUM") as ps_v, \
                     tc.tile_pool(name="sc_sc", bufs=2, space="PSUM") as ps_sc:
                    for b in range(NBLK):
                        hs = []
                        for mb in range(n_mb):
                            if mb == 0:
                                zsrc_ap = zs[:, b * BLK:(b + 1) * BLK]
                            else:
                                zt = sb.tile([128, BLK], BF16, tag="sp_zt")
                                nc.sync.dma_start(zt[:], zdram[:, b * BLK:(b + 1) * BLK])
                                zsrc_ap = zt[:]
                            h = sb.tile([128, BLK], BF16, tag=f"sp_h{mb}")
                            nc.scalar.activation(h[:], zsrc_ap, AFT.Relu,
                                                 bias=Cc[mb], scale=A[mb])
                            hs.append(h)
                        hE = sb.tile([128, NCHUNK * Cout], BF16, tag="sp_hE")
                        for ch in range(NCHUNK):
                            for mb in range(n_mb):
                                tp = ps_tp.tile([128, 128], BF16, tag="sp_tp", space="PSUM")
                                nc.tensor.transpose(tp[:], hs[mb][:, ch * 128:(ch + 1) * 128],
                                                    ident[:])
                                dst = hE[:, ch * Cout + mb * 128:ch * Cout + (mb + 1) * 128]
                                if (ch + mb) % 2 == 0:
                                    nc.vector.tensor_copy(dst, tp[:])
                                else:
                                    nc.scalar.copy(dst, tp[:])
                        for w in range(NW_BLK):
                            gw = b * NW_BLK + w
                            sc = ps_sc.tile([128, Cout], F32, tag="sp_sc", space="PSUM")
                            for cb in range(B):
                                ch = w * B + cb
                                col = b * NCHUNK + ch
                                oh = sb.tile([128, 128], BF16, tag="sp_oh")
                                nc.vector.tensor_tensor(
                                    out=oh[:],
                                    in0=dwinb[:, col:col + 1].to_broadcast([128, 128]),
                                    in1=iotab[:], op=AOP.is_equal)
                                nc.tensor.matmul(sc[:], oh[:],
                                                 hE[:, ch * Cout:(ch + 1) * Cout],
                                                 start=(cb == 0), stop=(cb == B - 1))
                            nt = sb.tile([128, Cout], BF16, tag="sp_nt")
                            nc.vector.tensor_scalar(nt[:], sc[:], invcnt[:, gw:gw + 1],
                                                    None, AOP.mult)
                            nc.sync.dma_start(tab_loc[gw * WIN:(gw + 1) * WIN, :], nt[:])
                            if vw is not None:
                                # V = nt @ W3b for the next conv's src side
                                ntT = sb.tile([128, 256], BF16, tag="sp_ntT")
                                for kb in range(2):
                                    tpv = ps_tp.tile([128, 128], BF16, tag="sp_tp",
                                                     space="PSUM")
                                    nc.tensor.transpose(
                                        tpv[:], nt[:, kb * 128:(kb + 1) * 128],
                                        ident[:])
                                    if kb == 0:
                                        nc.vector.tensor_copy(ntT[:, 0:128], tpv[:])
                                    else:
                                        nc.scalar.copy(ntT[:, 128:256], tpv[:])
                                vps = ps_v.tile([128, 256], F32, tag="sp_v",
                                                space="PSUM")
                                for kb in range(2):
                                    nc.tensor.matmul(vps[:],
                                                     ntT[:, kb * 128:(kb + 1) * 128],
                                                     vw[kb][:],
                                                     start=(kb == 0), stop=(kb == 1))
                                nv = sb.tile([128, 256], BF16, tag="sp_nv")
                                nc.vector.tensor_copy(nv[:], vps[:])
                                nc.sync.dma_start(vdst[gw * WIN:(gw + 1) * WIN, :],
                                                  nv[:])

            # ======================= CONV 1 =======================
            with tc.tile_pool(name="c1sb", bufs=2) as sb:
                c1b = [[load_vec(t_c1b[i], sb, f"c1b{i}")] for i in range(3)]
                c1gn = [[[load_vec(t_c1gn[i, j], sb, f"c1gn{i}{j}")] for j in range(3)]
                        for i in range(3)]
                with tc.tile_pool(name="c1big", bufs=2) as bp, \
                     tc.tile_pool(name="c1ps", bufs=2, space="PSUM") as ps:
                    c1w = []
                    for i in range(3):
                        w = sb.tile([128, 128], BF16, tag=f"c1w{i}")
                        nc.sync.dma_start(w[:], t_c1w[i])
                        c1w.append(w)

                    # layer 1: z1 -> zs (4 partition-quarters of msgT)
                    s1 = sb.tile([128, 1], F32, tag="s1")
                    q1 = sb.tile([128, 1], F32, tag="q1")
                    nc.vector.memset(s1[:], 0.0)
                    nc.vector.memset(q1[:], 0.0)
                    with tc.tile_pool(name="c1msg", bufs=1) as msp:
                        msgT = msp.tile([128, EQP], BF16, name="msgT")
                        nc.sync.dma_start(msgT[:], t_msgT[:])
                        for q in range(4):
                            for g in range(7):          # groups of 4 segs
                                s0 = g * 4
                                nseg = min(4, NSEGQ - s0)
                                zp4 = ps.tile([128, 2048], F32, tag="zp4")
                                for si in range(nseg):
                                    s = s0 + si
                                    nc.tensor.matmul(
                                        zp4[:, si * 512:(si + 1) * 512],
                                        c1w[0][32 * q:32 * q + 10, :],
                                        msgT[32 * q:32 * q + 10,
                                             s * 512:(s + 1) * 512],
                                        start=True, stop=True,
                                        tile_position=(32 * q, 0) if q == 3 else None)
                                col = q * EQ + s0 * 512
                                nv = min(nseg * 512, EQ - s0 * 512)
                                zsink(zp4[:, :nv], zs[:, col:col + nv],
                                      s1[:, 0:1], sb, par=g)
                    for b in range(NBLK):
                        blk_sq(zs[:, b * BLK:(b + 1) * BLK], q1[:, 0:1], sb, par=b)
                    stg = allgather_stats(s1, q1, 1, sb)
                    A1, C1 = affine_from_stats(stg, 1, c1b[0], c1gn[0], sb)

                    # layers 2+3: z = W @ relu(aff(z_prev)), in-place in zs
                    ls_params = []
                    for li, wt in ((1, c1w[1]), (2, c1w[2])):
                        AA, CC = (A1, C1) if li == 1 else ls_params[0]
                        sL = sb.tile([128, 1], F32, tag=f"s{li + 1}")
                        qL = sb.tile([128, 1], F32, tag=f"q{li + 1}")
                        nc.vector.memset(sL[:], 0.0)
                        nc.vector.memset(qL[:], 0.0)
                        for b in range(NBLK):
                            h1 = bp.tile([128, BLK], BF16, tag="h1")
                            nc.scalar.activation(h1[:], zs[:, b * BLK:(b + 1) * BLK],
                                                 AFT.Relu, bias=CC[0], scale=AA[0])
                            for g, nseg in ((0, 4), (1, 3)):
                                zp4 = ps.tile([128, 2048], F32, tag="zp4")
                                for si in range(nseg):
                                    s = g * 4 + si
                                    nc.tensor.matmul(zp4[:, si * 512:(si + 1) * 512],
                                                     wt[:],
                                                     h1[:, s * 512:(s + 1) * 512],
                                                     start=True, stop=True)
                                col = b * BLK + g * 4 * 512
                                nv = nseg * 512
                                zsink(zp4[:, :nv], zs[:, col:col + nv],
                                      sL[:, 0:1], sb, par=g)
                            blk_sq(zs[:, b * BLK:(b + 1) * BLK], qL[:, 0:1], sb,
                                   par=b)
                        zsent = sb.tile([128, 1], F32, tag=f"zsent{li}")
                        nc.vector.tensor_copy(zsent[:], zs[:, E_PAD - 1:E_PAD])
                        sentinel_correct(sL, qL, [zsent[:]], 1, sb)
                        stgL = allgather_stats(sL, qL, 1, sb)
                        AL, CL = affine_from_stats(stgL, 1, c1b[li], c1gn[li], sb)
                        ls_params = [(AL, CL)]
                    A3, C3 = ls_params[0]

                scatter_pass(None, 1, A3, C3, tab1_loc, 128)

            nc.gpsimd.collective_compute(
                "AllGather", AOP.bypass, replica_groups=[list(range(NC))],
                ins=[tab1_loc.opt()], outs=[tab1.opt()])
            if debug:
                nc.sync.dma_start(dbg["x1"][:], tab1[:])

            # ============== gather-based first layer (conv2/conv3) ==============
            def gather_layer(tab_full, tab_loc_src, Cin, wa_t, wb_t, n_kb, zdram, sb):
                """z = WA @ xi + WB @ xj per 512-edge window;
                mo=0 -> zs SBUF, mo=1 -> zdram (block-staged)."""
                mb_in = Cin // 128
                s_acc = sb.tile([128, 2], F32, tag="gl_s")
                q_acc = sb.tile([128, 2], F32, tag="gl_q")
                nc.vector.memset(s_acc[:], 0.0)
                nc.vector.memset(q_acc[:], 0.0)
                ones1 = sb.tile([1, 128], BF16, tag="gl_ones")
                nc.vector.memset(ones1[:], 1.0)
                with tc.tile_pool(name="gl_g2", bufs=2) as g2, \
                     tc.tile_pool(name="gl_zw", bufs=2) as zwp, \
                     tc.tile_pool(name="gl_ps", bufs=2, space="PSUM") as ps, \
                     tc.tile_pool(name="gl_tp", bufs=2, space="PSUM") as ps_tp, \
                     tc.tile_pool(name="gl_xp", bufs=2, space="PSUM") as ps_xp:
                    was, wbs = [], []
                    for ki in range(n_kb):
                        for mo in range(2):
                            wta = sb.tile([128, 128], BF16, tag=f"gl_wa{ki}{mo}")
                            nc.sync.dma_start(wta[:], wa_t[ki, mo] if n_kb > 1 else wa_t[mo])
                            was.append(wta)
                            wtb = sb.tile([128, 128], BF16, tag=f"gl_wb{ki}{mo}")
                            nc.sync.dma_start(wtb[:], wb_t[ki, mo] if n_kb > 1 else wb_t[mo])
                            wbs.append(wtb)
                    for b in range(NBLK):
                        ixj = g2.tile([128, NCHUNK], mybir.dt.int32, tag="gl_ixj")
                        nc.sync.dma_start(ixj[:], t_xj[:, b * NCHUNK:(b + 1) * NCHUNK])
                        dwr = g2.tile([1, BLK], BF16, tag="gl_dwr")
                        nc.sync.dma_start(dwr[:], t_dwrow[0:1, b * BLK:(b + 1) * BLK])
                        zw = zwp.tile([128, BLK], BF16, tag="gl_zw")
                        for w in range(NW_BLK):
                            gw = b * NW_BLK + w
                            twin = g2.tile([128, Cin], BF16, tag="gl_twin")
                            nc.sync.dma_start(twin[:],
                                              tab_loc_src[gw * WIN:(gw + 1) * WIN, :])
                            # one-hot rows oh2[s, e] = (dstwin[e] == s) for the
                            # window's 512 edges, via K=1 broadcast matmul
                            bc = ps_xp.tile([128, 512], F32, tag="gl_bc", space="PSUM")
                            nc.tensor.matmul(bc[:], ones1[:],
                                             dwr[0:1, w * 512:(w + 1) * 512],
                                             start=True, stop=True)
                            oh2 = g2.tile([128, 512], BF16, tag="gl_oh2")
                            nc.vector.tensor_tensor(
                                out=oh2[:], in0=bc[:],
                                in1=iotap[:].to_broadcast([128, 512]),
                                op=AOP.is_equal)
                            # xi feature-major via twin.T @ oh2
                            xiT = g2.tile([128, mb_in * 512], BF16, tag="gl_xiT")
                            for kb in range(mb_in):
                                xp = ps_xp.tile([128, 512], F32, tag="gl_xp",
                                                space="PSUM")
                                nc.tensor.matmul(
                                    xp[:], twin[:, kb * 128:(kb + 1) * 128],
                                    oh2[:], start=True, stop=True)
                                nc.vector.tensor_copy(
                                    xiT[:, kb * 512:(kb + 1) * 512], xp[:])
                            # xj gather + transpose to feature-major
                            gxj = g2.tile([128, B * Cin], BF16, tag="gl_gxj")
                            for cb in range(B):
                                ch = w * B + cb
                                nc.gpsimd.indirect_dma_start(
                                    out=gxj[:, cb * Cin:(cb + 1) * Cin],
                                    out_offset=None,
                                    in_=tab_full[:],
                                    in_offset=bass.IndirectOffsetOnAxis(
                                        ap=ixj[:, ch:ch + 1], axis=0))
                            xjT = g2.tile([128, mb_in * 512], BF16, tag="gl_xjT")
                            for cb in range(B):
                                for kb in range(mb_in):
                                    tp2 = ps_tp.tile([128, 128], BF16, tag="gl_tp2",
                                                     space="PSUM")
                                    nc.tensor.transpose(
                                        tp2[:],
                                        gxj[:, cb * Cin + kb * 128:cb * Cin + (kb + 1) * 128],
                                        ident[:])
                                    nc.vector.tensor_copy(
                                        xjT[:, kb * 512 + cb * 128:kb * 512 + (cb + 1) * 128],
                                        tp2[:])
                            # z for this window's 512 edges
                            for mo in range(2):
                                zp = ps.tile([128, 512], F32, tag="gl_zp")
                                for ki in range(mb_in):
                                    nc.tensor.matmul(
                                        zp[:], was[ki * 2 + mo][:],
                                        xiT[:, ki * 512:(ki + 1) * 512],
                                        start=(ki == 0), stop=False)
                                for ki in range(mb_in):
                                    nc.tensor.matmul(
                                        zp[:], wbs[ki * 2 + mo][:],
                                        xjT[:, ki * 512:(ki + 1) * 512],
                                        start=False, stop=(ki == mb_in - 1))
                                if mo == 0:
                                    col = b * BLK + w * 512
                                    zsink(zp[:], zs[:, col:col + 512],
                                          s_acc[:, 0:1], sb, par=w + mo)
                                else:
                                    zsink(zp[:], zw[:, w * 512:(w + 1) * 512],
                                          s_acc[:, 1:2], sb, par=w + mo)
                        nc.sync.dma_start(zdram[:, b * BLK:(b + 1) * BLK], zw[:])
                        blk_sq(zs[:, b * BLK:(b + 1) * BLK], q_acc[:, 0:1], sb,
                               par=b)
                        blk_sq(zw[:], q_acc[:, 1:2], sb, par=b + 1)
                return s_acc, q_acc

            def gather_layer_v(vtab_full, tab_loc_src, wa_t, zmid, zout, sb):
                """conv3: z = WA @ xi + V[src]; two phases.
                Phase A computes the xi-side partial z (no dep on the V
                AllGather, so it overlaps it); phase B gathers V rows and
                transpose-accumulates them via identity matmuls."""
                Cin = 256
                mb_in = 2
                s_acc = sb.tile([128, 2], F32, tag="gv_s")
                q_acc = sb.tile([128, 2], F32, tag="gv_q")
                nc.vector.memset(s_acc[:], 0.0)
                nc.vector.memset(q_acc[:], 0.0)
                ones1 = sb.tile([1, 128], BF16, tag="gv_ones")
                nc.vector.memset(ones1[:], 1.0)
                was = []
                for ki in range(2):
                    for mo in range(2):
                        wta = sb.tile([128, 128], BF16, tag=f"gv_wa{ki}{mo}")
                        nc.sync.dma_start(wta[:], wa_t[ki, mo])
                        was.append(wta)
                # Phase A: xi partials -> zs (mo0) / zmid (mo1)
                with tc.tile_pool(name="ga_g2", bufs=2) as g2, \
                     tc.tile_pool(name="ga_zw", bufs=2) as zwp, \
                     tc.tile_pool(name="ga_ps", bufs=2, space="PSUM") as ps, \
                     tc.tile_pool(name="ga_xp", bufs=2, space="PSUM") as ps_xp:
                    for b in range(NBLK):
                        dwr = g2.tile([1, BLK], BF16, tag="ga_dwr")
                        nc.sync.dma_start(dwr[:], t_dwrow[0:1, b * BLK:(b + 1) * BLK])
                        zwa = zwp.tile([128, BLK], BF16, tag="ga_zwa")
                        for w in range(NW_BLK):
                            gw = b * NW_BLK + w
                            twin = g2.tile([128, Cin], BF16, tag="ga_twin")
                            nc.sync.dma_start(twin[:],
                                              tab_loc_src[gw * WIN:(gw + 1) * WIN, :])
                            bc = ps_xp.tile([128, 512], F32, tag="ga_bc", space="PSUM")
                            nc.tensor.matmul(bc[:], ones1[:],
                                             dwr[0:1, w * 512:(w + 1) * 512],
                                             start=True, stop=True)
                            oh2 = g2.tile([128, 512], BF16, tag="ga_oh2")
                            nc.vector.tensor_tensor(
                                out=oh2[:], in0=bc[:],
                                in1=iotap[:].to_broadcast([128, 512]),
                                op=AOP.is_equal)
                            xiT = g2.tile([128, mb_in * 512], BF16, tag="ga_xiT")
                            for kb in range(mb_in):
                                xp = ps_xp.tile([128, 512], F32, tag="ga_xp",
                                                space="PSUM")
                                nc.tensor.matmul(
                                    xp[:], twin[:, kb * 128:(kb + 1) * 128],
                                    oh2[:], start=True, stop=True)
                                nc.vector.tensor_copy(
                                    xiT[:, kb * 512:(kb + 1) * 512], xp[:])
                            for mo in range(2):
                                zp = ps.tile([128, 512], F32, tag="ga_zp")
                                for ki in range(mb_in):
                                    nc.tensor.matmul(
                                        zp[:], was[ki * 2 + mo][:],
                                        xiT[:, ki * 512:(ki + 1) * 512],
                                        start=(ki == 0), stop=(ki == mb_in - 1))
                                if mo == 0:
                                    col = b * BLK + w * 512
                                    nc.scalar.copy(zs[:, col:col + 512], zp[:])
                                else:
                                    nc.scalar.copy(zwa[:, w * 512:(w + 1) * 512],
                                                   zp[:])
                        nc.sync.dma_start(zmid[:, b * BLK:(b + 1) * BLK], zwa[:])
                # Phase B: V rows, transpose-accumulate via identity matmul
                with tc.tile_pool(name="gb_g2", bufs=2) as g2, \
                     tc.tile_pool(name="gb_zw", bufs=2) as zwp, \
                     tc.tile_pool(name="gb_ps", bufs=2, space="PSUM") as ps:
                    for b in range(NBLK):
                        ixj = g2.tile([128, NCHUNK], mybir.dt.int32, tag="gb_ixj")
                        nc.sync.dma_start(ixj[:], t_xj[:, b * NCHUNK:(b + 1) * NCHUNK])
                        zxw = zwp.tile([128, BLK], BF16, tag="gb_zxw")
                        nc.sync.dma_start(zxw[:], zmid[:, b * BLK:(b + 1) * BLK])
                        zw = zwp.tile([128, BLK], BF16, tag="gb_zw")
                        for w in range(NW_BLK):
                            gv = g2.tile([128, B * 256], BF16, tag="gb_gv")
                            for cb in range(B):
                                ch = w * B + cb
                                nc.gpsimd.indirect_dma_start(
                                    out=gv[:, cb * 256:(cb + 1) * 256],
                                    out_offset=None,
                                    in_=vtab_full[:],
                                    in_offset=bass.IndirectOffsetOnAxis(
                                        ap=ixj[:, ch:ch + 1], axis=0))
                            for mo in range(2):
                                zp = ps.tile([128, 512], F32, tag="gb_zp")
                                for cb in range(B):
                                    nc.tensor.matmul(
                                        zp[:, cb * 128:(cb + 1) * 128],
                                        gv[:, cb * 256 + mo * 128:
                                           cb * 256 + (mo + 1) * 128],
                                        ident[:], start=True, stop=True)
                                sa = sb.tile([128, 1], F32, tag="gb_sa")
                                col = b * BLK + w * 512
                                if mo == 0:
                                    nc.vector.tensor_tensor(
                                        zs[:, col:col + 512],
                                        zs[:, col:col + 512], zp[:], op=AOP.add)
                                    nc.vector.reduce_sum(
                                        out=sa[:], in_=zs[:, col:col + 512],
                                        axis=AX.X)
                                else:
                                    nc.vector.tensor_tensor(
                                        zw[:, w * 512:(w + 1) * 512],
                                        zxw[:, w * 512:(w + 1) * 512], zp[:],
                                        op=AOP.add)
                                    nc.vector.reduce_sum(
                                        out=sa[:], in_=zw[:, w * 512:(w + 1) * 512],
                                        axis=AX.X)
                                nc.vector.tensor_tensor(s_acc[:, mo:mo + 1],
                                                        s_acc[:, mo:mo + 1],
                                                        sa[:], op=AOP.add)
                        nc.sync.dma_start(zout[:, b * BLK:(b + 1) * BLK], zw[:])
                        blk_sq(zs[:, b * BLK:(b + 1) * BLK], q_acc[:, 0:1], sb,
                               par=b)
                        blk_sq(zw[:], q_acc[:, 1:2], sb, par=b + 1)
                return s_acc, q_acc

            # ======================= CONV 2 =======================
            if phases >= 2:
              with tc.tile_pool(name="c2sb", bufs=2) as sb:
                  c2b = [[load_vec(t_c2b[i, mb], sb, f"c2b{i}{mb}") for mb in range(2)]
                         for i in range(2)]
                  c2gn = [[[load_vec(t_c2gn[i, j, mb], sb, f"c2gn{i}{j}{mb}")
                            for mb in range(2)] for j in range(3)] for i in range(2)]
                  sA, qA = gather_layer(tab1, tab1_loc, 128, t_c2wa, t_c2wb, 1,
                                        z_scr[0], sb)
                  stg = allgather_stats(sA, qA, 2, sb)
                  A1, C1 = affine_from_stats(stg, 2, c2b[0], c2gn[0], sb)

                  # layer 2: z2 = W2 @ relu(aff(zA)); mb0 in-place zs, mb1 -> z_scr[1]
                  s2 = sb.tile([128, 2], F32, tag="c2s2")
                  q2 = sb.tile([128, 2], F32, tag="c2q2")
                  nc.vector.memset(s2[:], 0.0)
                  nc.vector.memset(q2[:], 0.0)
                  zsent = []
                  with tc.tile_pool(name="c2mid", bufs=2) as mp, \
                       tc.tile_pool(name="c2ps", bufs=2, space="PSUM") as ps:
                      w2s = []
                      for ki in range(2):
                          for mo in range(2):
                              w = sb.tile([128, 128], BF16, tag=f"c2w2{ki}{mo}")
                              nc.sync.dma_start(w[:], t_c2w2[ki, mo])
                              w2s.append(w)
                      for b in range(NBLK):
                          h1 = []
                          z1d = mp.tile([128, BLK], BF16, tag="c2z1r")
                          nc.sync.dma_start(z1d[:], z_scr[0][:, b * BLK:(b + 1) * BLK])
                          for mb in range(2):
                              zsrc_ap = (zs[:, b * BLK:(b + 1) * BLK] if mb == 0
                                         else z1d[:])
                              hh = mp.tile([128, BLK], BF16, tag=f"c2h1{mb}")
                              nc.scalar.activation(hh[:], zsrc_ap, AFT.Relu,
                                                   bias=C1[mb], scale=A1[mb])
                              h1.append(hh)
                          zw = mp.tile([128, BLK], BF16, tag="c2zw")
                          for mo in range(2):
                              for g, nseg in ((0, 4), (1, 3)):
                                  zp4 = ps.tile([128, 2048], F32, tag="c2zp4")
                                  for si in range(nseg):
                                      s = g * 4 + si
                                      for ki in range(2):
                                          nc.tensor.matmul(
                                              zp4[:, si * 512:(si + 1) * 512],
                                              w2s[ki * 2 + mo][:],
                                              h1[ki][:, s * 512:(s + 1) * 512],
                                              start=(ki == 0), stop=(ki == 1))
                                  col = g * 4 * 512
                                  nv = nseg * 512
                                  if mo == 0:
                                      zsink(zp4[:, :nv],
                                            zs[:, b * BLK + col:b * BLK + col + nv],
                                            s2[:, 0:1], sb, par=g + mo)
                                  else:
                                      zsink(zp4[:, :nv], zw[:, col:col + nv],
                                            s2[:, 1:2], sb, par=g + mo)
                          nc.sync.dma_start(z_scr[1][:, b * BLK:(b + 1) * BLK], zw[:])
                          blk_sq(zs[:, b * BLK:(b + 1) * BLK], q2[:, 0:1], sb, par=b)
                          blk_sq(zw[:], q2[:, 1:2], sb, par=b + 1)
                          if b == NBLK - 1:
                              zc0 = sb.tile([128, 1], F32, tag="c2zs0")
                              nc.vector.tensor_copy(zc0[:], zs[:, E_PAD - 1:E_PAD])
                              zc1 = sb.tile([128, 1], F32, tag="c2zs1")
                              nc.vector.tensor_copy(zc1[:], zw[:, BLK - 1:BLK])
                              zsent = [zc0[:], zc1[:]]
                  sentinel_correct(s2, q2, zsent, 2, sb)
                  stg2 = allgather_stats(s2, q2, 2, sb)
                  A2, C2 = affine_from_stats(stg2, 2, c2b[1], c2gn[1], sb)
                  vwt = []
                  for kb in range(2):
                      w = sb.tile([128, 256], BF16, tag=f"c2vw{kb}")
                      nc.sync.dma_start(w[:], t_c3wbv[kb])
                      vwt.append(w)
                  scatter_pass(z_scr[1], 2, A2, C2, tab2_loc, 256,
                               vw=vwt, vdst=vt_loc)

            nc.gpsimd.collective_compute(
                "AllGather", AOP.bypass, replica_groups=[list(range(NC))],
                ins=[vt_loc.opt()], outs=[vtab.opt()])
            if debug:
                nc.gpsimd.collective_compute(
                    "AllGather", AOP.bypass, replica_groups=[list(range(NC))],
                    ins=[tab2_loc.opt()], outs=[tab2.opt()])
                nc.sync.dma_start(dbg["x2"][:], tab2[:])

            # ======================= CONV 3 =======================
            if phases >= 3:
              with tc.tile_pool(name="c3sb", bufs=2) as sb:
                  c3b = [load_vec(t_c3b[mb], sb, f"c3b{mb}") for mb in range(2)]
                  c3gn = [[load_vec(t_c3gn[j, mb], sb, f"c3gn{j}{mb}") for mb in range(2)]
                          for j in range(3)]
                  sA, qA = gather_layer_v(vtab, tab2_loc, t_c3wa,
                                          z_scr[1], z_scr[0], sb)
                  stg = allgather_stats(sA, qA, 2, sb)
                  A1, C1 = affine_from_stats(stg, 2, c3b, c3gn, sb)
                  scatter_pass(z_scr[0], 2, A1, C1, tab3_loc, 256)

            if debug:
                nc.sync.dma_start(dbg["x3"][:], tab3_loc[:])

            # ======================= POOL + HEAD =======================
            if phases >= 4:
              with tc.tile_pool(name="p_sb", bufs=2) as sb, \
                 tc.tile_pool(name="p_ps", bufs=2, space="PSUM") as ps:
                  pgwl = sb.tile([128, 8 * Bg], F32, tag="p_pgwl")
                  nc.sync.dma_start(pgwl[:], t_pgwl[:])
                  for gw in range(8):
                      pidx = sb.tile([128, Bg], mybir.dt.int32, tag="p_idx")
                      nc.sync.dma_start(pidx[:], t_pidx[gw])
                      gp = sb.tile([128, Bg * 256], BF16, tag="p_gp")
                      for c in range(Bg):
                          nc.gpsimd.indirect_dma_start(
                              out=gp[:, c * 256:(c + 1) * 256], out_offset=None,
                              in_=tab3_loc[:],
                              in_offset=bass.IndirectOffsetOnAxis(
                                  ap=pidx[:, c:c + 1], axis=0))
                      pp = ps.tile([128, 256], F32, tag="p_pp", space="PSUM")
                      for c in range(Bg):
                          oh = sb.tile([128, 128], BF16, tag="p_oh")
                          nc.vector.tensor_tensor(
                              out=oh[:],
                              in0=pgwl[:, gw * Bg + c:gw * Bg + c + 1].to_broadcast([128, 128]),
                              in1=iota[:], op=AOP.is_equal)
                          nc.tensor.matmul(pp[:], oh[:], gp[:, c * 256:(c + 1) * 256],
                                           start=(c == 0), stop=(c == Bg - 1))
                      pf = sb.tile([128, 256], F32, tag="p_pf")
                      nc.vector.tensor_copy(pf[:], pp[:])
                      nc.sync.dma_start(pool_in[gw * 128:(gw + 1) * 128, :], pf[:])
                  nc.gpsimd.collective_compute(
                      "ReduceScatter", AOP.add, replica_groups=[list(range(NC))],
                      ins=[pool_in.opt()], outs=[pool_rs.opt()])
                  if debug:
                      nc.sync.dma_start(dbg["pool"][:], pool_rs[:])

                  invg = sb.tile([128, 1], F32, tag="p_invg")
                  nc.sync.dma_start(invg[:], t_invg[:])
                  lw1 = []
                  for ki in range(2):
                      for mo in range(2):
                          w = sb.tile([128, 128], BF16, tag=f"p_lw1{ki}{mo}")
                          nc.sync.dma_start(w[:], t_lw1[ki, mo])
                          lw1.append(w)
                  lw2 = []
                  for ki in range(2):
                      w = sb.tile([128, 2], BF16, tag=f"p_lw2{ki}")
                      nc.sync.dma_start(w[:], t_lw2[ki])
                      lw2.append(w)
                  lb1 = [load_vec(t_lb1[mb], sb, f"p_lb1{mb}") for mb in range(2)]
                  lb2 = sb.tile([2, 1], F32, tag="p_lb2")
                  nc.sync.dma_start(lb2[:], t_lb2[:])
                  g = sb.tile([128, 256], F32, tag="p_g")
                  nc.sync.dma_start(g[:], pool_rs[:])
                  gm = sb.tile([128, 256], BF16, tag="p_gm")
                  nc.vector.tensor_scalar(gm[:], g[:], invg[:, 0:1], None, AOP.mult)
                  gT = sb.tile([128, 2 * 128], BF16, tag="p_gT")
                  for kb in range(2):
                      tp = ps.tile([128, 128], BF16, tag="p_tp", space="PSUM")
                      nc.tensor.transpose(tp[:], gm[:, kb * 128:(kb + 1) * 128], ident[:])
                      nc.vector.tensor_copy(gT[:, kb * 128:(kb + 1) * 128], tp[:])
                  hT = sb.tile([128, 2 * 128], BF16, tag="p_hT")
                  for mo in range(2):
                      hp = ps.tile([128, 128], F32, tag="p_hp", space="PSUM")
                      for ki in range(2):
                          nc.tensor.matmul(hp[:], lw1[ki * 2 + mo][:],
                                           gT[:, ki * 128:(ki + 1) * 128],
                                           start=(ki == 0), stop=(ki == 1))
                      nc.scalar.activation(hT[:, mo * 128:(mo + 1) * 128], hp[:],
                                           AFT.Relu, bias=lb1[mo])
                  op_ = ps.tile([2, 128], F32, tag="p_op", space="PSUM")
                  for ki in range(2):
                      nc.tensor.matmul(op_[:], lw2[ki][:],
                                       hT[:, ki * 128:(ki + 1) * 128],
                                       start=(ki == 0), stop=(ki == 1))
                  ofin = sb.tile([2, 128], F32, tag="p_out")
                  nc.vector.tensor_scalar(ofin[:], op_[:], lb2[:], None, AOP.add)
                  nc.sync.dma_start(o_out[:], ofin[:])

    nc.compile()
    return nc


# ============================ entry point ============================


def kernel(**inputs):
    x = np.asarray(inputs["x"], dtype=np.float32)
    edge_index = np.asarray(inputs["edge_index"])
    batch = np.asarray(inputs["batch"])

    meta = _pack(edge_index, batch)
    Bg = meta["Bg"]

    import os as _os
    phases = int(_os.environ.get("KPHASES", "4"))
    key = ("mod", Bg, phases, _DEBUG[0])
    if key not in _cache:
        _cache[key] = _build(Bg, debug=bool(inputs.get("_debug", False)) or _DEBUG[0],
                             phases=phases)
    nc = _cache[key]

    # ---- per-core input arrays ----
    src = np.asarray(edge_index[0], dtype=np.int64)
    dst = np.asarray(edge_index[1], dtype=np.int64)

    # conv1 msgT: [core, 128, EQP] bf16; quarter q of the edge range lives on
    # partition rows 32q..32q+10, cols 0..EQ (padded to EQP with zeros)
    xi_v = x[dst]
    xj_v = x[src]
    msg = np.concatenate([xi_v, xj_v - xi_v], axis=1)       # [E, 10]
    msg_full = np.zeros((NC, E_PAD, 10), dtype=np.float32)
    ec, pos = meta["ec"], meta["pos"]
    msg_full[ec, pos] = msg[meta["eorder"]]
    msgT = np.zeros((NC, 128, EQP), dtype=ml_dtypes.bfloat16)
    for q in range(4):
        msgT[:, 32 * q:32 * q + 10, :EQ] = _bf(
            msg_full[:, q * EQ:(q + 1) * EQ].transpose(0, 2, 1))

    dstwin = meta["dstwin"]  # [NC, E_PAD]
    dwin_in = np.ascontiguousarray(
        dstwin.reshape(NC, E_PAD // 128, 128).transpose(0, 2, 1)).astype(np.float32)
    invcnt_in = np.ascontiguousarray(
        meta["inv_cnt"].reshape(NC, NWIN, 128).transpose(0, 2, 1)).astype(np.float32)
    padcnt_in = np.repeat(meta["pad_cnt"][:, None], 128, axis=1)[:, :, None].astype(np.float32)

    iota_in = np.broadcast_to(np.arange(128, dtype=np.float32)[None, :], (128, 128))
    iota_in = np.ascontiguousarray(iota_in)
    iotap_in = np.arange(128, dtype=np.float32).reshape(128, 1)
    ident_in = np.eye(128, dtype=np.float32).astype(ml_dtypes.bfloat16)
    dwrow_in = dstwin.astype(ml_dtypes.bfloat16).reshape(NC, 1, E_PAD)

    xj_in = np.ascontiguousarray(
        meta["xj_glob"].reshape(NC, E_PAD // 128, 128).transpose(0, 2, 1)).astype(np.int32)

    # weights
    c1w = np.zeros((3, 128, 128), dtype=ml_dtypes.bfloat16)
    for q in range(4):
        c1w[0, 32 * q:32 * q + 10, :] = _bf(inputs["c1_w1"])
    c1w[1] = _bf(inputs["c1_w2"])
    c1w[2] = _bf(inputs["c1_w3"])
    c1b = np.stack([np.asarray(inputs[f"c1_b{i}"], dtype=np.float32).reshape(128, 1)
                    for i in (1, 2, 3)])
    c1gn = np.stack([np.asarray(inputs[f"c1_gn{i}"], dtype=np.float32).reshape(3, 128, 1)
                     for i in (1, 2, 3)])

    w2a = np.asarray(inputs["c2_w1"], dtype=np.float32)   # [256, 256]
    WA2 = w2a[:128] - w2a[128:]
    WB2 = w2a[128:]
    c2wa = _tile_w(WA2)[0]                                # [2, 128, 128]
    c2wb = _tile_w(WB2)[0]
    c2w2 = _tile_w(np.asarray(inputs["c2_w2"], dtype=np.float32))  # [2,2,128,128]
    c2b = np.stack([np.asarray(inputs["c2_b1"], dtype=np.float32).reshape(2, 128, 1),
                    np.asarray(inputs["c2_b2"], dtype=np.float32).reshape(2, 128, 1)])
    c2gn = np.stack([np.asarray(inputs["c2_gn1"], dtype=np.float32).reshape(3, 2, 128, 1),
                     np.asarray(inputs["c2_gn2"], dtype=np.float32).reshape(3, 2, 128, 1)])

    w3a = np.asarray(inputs["c3_w1"], dtype=np.float32)   # [512, 256]
    WA3 = w3a[:256] - w3a[256:]
    WB3 = w3a[256:]
    c3wa = _tile_w(WA3)                                   # [2,2,128,128]
    c3wbv = np.stack([_bf(WB3[:128]), _bf(WB3[128:])])    # [2,128,256]
    c3b = np.asarray(inputs["c3_b1"], dtype=np.float32).reshape(2, 128, 1)
    c3gn = np.asarray(inputs["c3_gn1"], dtype=np.float32).reshape(3, 2, 128, 1)

    lw1 = _tile_w(np.asarray(inputs["lin_w1"], dtype=np.float32))
    lb1 = np.asarray(inputs["lin_b1"], dtype=np.float32).reshape(2, 128, 1)
    lw2_f = np.asarray(inputs["lin_w2"], dtype=np.float32)  # [256, 2]
    lw2 = np.stack([_bf(lw2_f[:128]), _bf(lw2_f[128:])])    # [2, 128, 2]
    lb2 = np.asarray(inputs["lin_b2"], dtype=np.float32).reshape(2, 1)

    Bg0 = meta["Bg"]
    pidx_in = np.ascontiguousarray(
        meta["pool_idx"].astype(np.int32).reshape(NC, 8, Bg0, 128).transpose(0, 1, 3, 2))
    pgwl = meta["pool_gwl"]                # [NC, 8, NPG]
    Bg_ = meta["Bg"]
    pgwl_in = np.ascontiguousarray(
        pgwl.reshape(NC, 8, Bg_, 128).transpose(0, 3, 1, 2)).reshape(NC, 128, 8 * Bg_)
    # per-core shard of 1/graph-count (graphs c*128 + p)
    invg_in = meta["inv_g"].reshape(8, 128)[:, :, None].astype(np.float32)

    in_maps = []
    for c in range(NC):
        im = {
            "msgT": msgT[c],
            "xj_idx": xj_in[c],
            "dstwin": dwin_in[c],
            "dwrow": np.ascontiguousarray(dwrow_in[c]),
            "invcnt": invcnt_in[c],
            "padcnt": padcnt_in[c],
            "iota": iota_in,
            "iotap": iotap_in,
            "ident": ident_in,
            "c1w": c1w, "c1b": c1b, "c1gn": c1gn,
            "c2wa": c2wa, "c2wb": c2wb, "c2w2": c2w2, "c2b": c2b, "c2gn": c2gn,
            "c3wa": c3wa, "c3wbv": c3wbv, "c3b": c3b, "c3gn": c3gn,
            "lw1": lw1, "lb1": lb1, "lw2": lw2, "lb2": lb2,
            "pool_idx": pidx_in[c],
            "pool_gwl": pgwl_in[c].astype(np.float32),
            "invg": np.ascontiguousarray(invg_in[c]),
        }
        in_maps.append(im)

    res = run_bass_kernel_spmd(nc, in_maps, core_ids=list(range(NC)),
                               trace=_TRACE[0])
    kernel.last_result = res
    kernel.last_meta = meta
    out = np.concatenate([res.results[c]["out"] for c in range(NC)], axis=1)
    return np.ascontiguousarray(out[:, :N_GRAPHS].T).astype(np.float32)


_DEBUG = [False]
_TRACE = [False]


# revision 40
# speedup vs baseline: 1.2182x; 1.2182x over previous
"""LundNetTagger GNN on 8 Trainium2 NeuronCores (Bass/Tile).

Self-contained: kernel(**inputs) -> np.ndarray [1000, 2] float32.

Strategy: nodes are assigned to 100352 "slots" (8 cores x 98 windows x 128),
packed so each window receives <= 512 edges. Edges live on the core owning
their dst slot, in window-major order padded to 4x128-edge chunks per window.
Per-edge MLPs run in bf16 feature-major layout; EdgeConv cat[xi, xj-xi] is
folded into split weights WA = W[:C]-W[C:], WB = W[C:]. GraphNorm stats are
global AllGathers of per-core sums. Mean-aggregation is a collision-free
one-hot matmul scatter into PSUM per window. Node tables are AllGathered in
bf16 (Shared outputs) between convs; src-side gathers use indirect DMA.
Intermediate z activations live in a persistent SBUF buffer (mb0) with only
the mb1 half round-tripping DRAM for the 256-wide convs.
"""
import numpy as np
import ml_dtypes

import concourse.bass as bass
import concourse.tile as tile
from concourse import bacc, mybir
from concourse.bass_utils import run_bass_kernel_spmd

BF16 = mybir.dt.bfloat16
F32 = mybir.dt.float32
AOP = mybir.AluOpType
AFT = mybir.ActivationFunctionType
AX = mybir.AxisListType

N_NODES = 100000
N_EDGES = 400000
N_GRAPHS = 1000
NC = 8
WIN = 128
NWIN = 98
SPC = WIN * NWIN          # 12544
NSLOTS = SPC * NC         # 100352
QUAD = NSLOTS // 4        # 25088
B = 4                     # chunks per window
EPW = B * WIN             # 512
E_PAD = NWIN * EPW        # 50176
EPS = 1e-5

NW_BLK = 7
BLK = NW_BLK * EPW        # 3584
NBLK = NWIN // NW_BLK     # 14
NCHUNK = BLK // 128       # 28
NSEG = BLK // 512         # 7


_cache = {}


# ============================ host-side packing ============================

def _pack(edge_index, batch):
    src = np.asarray(edge_index[0], dtype=np.int64)
    dst = np.asarray(edge_index[1], dtype=np.int64)
    batch = np.asarray(batch, dtype=np.int64)
    cnt = np.bincount(dst, minlength=N_NODES)

    nvirt = NSLOTS - N_NODES
    cnt_all = np.concatenate([cnt, np.zeros(nvirt, dtype=cnt.dtype)])
    order = np.argsort(-cnt_all, kind="stable")
    GW = NWIN * NC
    rounds = NSLOTS // GW
    win_of_rank = np.empty(NSLOTS, dtype=np.int64)
    for r in range(rounds):
        seg = np.arange(GW) if r % 2 == 0 else np.arange(GW - 1, -1, -1)
        win_of_rank[r * GW:(r + 1) * GW] = seg
    win_of_node = np.empty(NSLOTS, dtype=np.int64)
    win_of_node[order] = win_of_rank
    wsum = np.bincount(win_of_node, weights=cnt_all.astype(np.float64),
                       minlength=GW).astype(np.int64)

    cap = EPW
    members_of = [list(np.where(win_of_node == w)[0]) for w in range(GW)]
    for _ in range(2000):
        over = np.where(wsum > cap)[0]
        if len(over) == 0:
            break
        w = int(over[0])
        # smallest-count >0 node in w
        mem = members_of[w]
        cs = [(int(cnt_all[n]), n) for n in mem if cnt_all[n] > 0]
        cs.sort()
        moved = False
        for c1, n in cs:
            # find target window with a smaller-count node to swap
            worder2 = np.argsort(wsum)
            for tw in worder2[:64]:
                tw = int(tw)
                if tw == w:
                    continue
                tmem = members_of[tw]
                best = None
                for m in tmem:
                    c2 = int(cnt_all[m])
                    if c2 < c1 and wsum[tw] + c1 - c2 <= cap:
                        if best is None or c2 < best[0]:
                            best = (c2, m)
                        if c2 == 0:
                            break
                if best is not None:
                    c2, m = best
                    members_of[tw].remove(m)
                    members_of[tw].append(n)
                    members_of[w].remove(n)
                    members_of[w].append(m)
                    win_of_node[n] = tw
                    win_of_node[m] = w
                    wsum[tw] += c1 - c2
                    wsum[w] -= c1 - c2
                    moved = True
                    break
            if moved:
                break
        if not moved:
            raise RuntimeError("packing fixup stuck")
    assert wsum.max() <= cap, f"window packing failed: max={wsum.max()}"

    worder = np.argsort(-wsum, kind="stable")
    core_load = np.zeros(NC, dtype=np.int64)
    core_nwin = np.zeros(NC, dtype=np.int64)
    core_of_win = np.empty(GW, dtype=np.int64)
    for w in worder:
        cands = np.where(core_nwin < NWIN)[0]
        c = cands[np.argmin(core_load[cands])]
        core_of_win[w] = c
        core_load[c] += wsum[w]
        core_nwin[c] += 1

    win_lists = [[] for _ in range(NC)]
    for w in range(GW):
        win_lists[core_of_win[w]].append(w)
    for c in range(NC):
        wl = win_lists[c]
        j = int(np.argmin(wsum[wl]))
        assert wsum[wl[j]] < cap, "no sentinel room"
        wl[j], wl[-1] = wl[-1], wl[j]

    slot_of_node = np.empty(NSLOTS, dtype=np.int64)
    for c in range(NC):
        for wi, w in enumerate(win_lists[c]):
            mem = np.sort(np.array(members_of[w], dtype=np.int64))
            assert len(mem) == WIN
            slot_of_node[mem] = c * SPC + wi * WIN + np.arange(WIN)
    node_of_slot = np.empty(NSLOTS, dtype=np.int64)
    node_of_slot[slot_of_node] = np.arange(NSLOTS)
    cnt_of_slot = cnt_all[node_of_slot]

    qzero = []
    for q in range(4):
        z = np.where(cnt_of_slot[q * QUAD:(q + 1) * QUAD] == 0)[0]
        assert len(z) > 0
        qzero.append(int(z[0]))  # local to quadrant
    czero = []
    for c in range(NC):
        z = np.where(cnt_of_slot[c * SPC:(c + 1) * SPC] == 0)[0]
        assert len(z) > 0
        czero.append(int(z[0]))  # local to core

    dslot = slot_of_node[dst]
    sslot = slot_of_node[src]
    ecore = dslot // SPC
    ewin = (dslot % SPC) // WIN
    key = ecore * (NWIN * WIN) + ewin * WIN + (dslot % WIN)
    eorder = np.argsort(key, kind="stable")
    dsl, ssl = dslot[eorder], sslot[eorder]
    ec, ew = ecore[eorder], ewin[eorder]

    cw = ec * NWIN + ew
    cw_cnt = np.bincount(cw, minlength=NC * NWIN)
    assert cw_cnt.max() <= EPW

    xi_idx = np.zeros((NC, E_PAD), dtype=np.int64)
    xj_idx = np.zeros((NC, E_PAD), dtype=np.int64)
    dstwin = np.full((NC, E_PAD), -1.0, dtype=np.float32)
    valid = np.zeros((NC, E_PAD), dtype=bool)

    ofs = (np.arange(NC * NWIN) % NWIN) * EPW
    start = np.concatenate([[0], np.cumsum(cw_cnt)[:-1]])
    within = np.arange(N_EDGES) - start[cw]
    pos = ofs[cw] + within
    xi_idx[ec, pos] = dsl % SPC
    xj_idx[ec, pos] = ssl
    dstwin[ec, pos] = (dsl % WIN).astype(np.float32)
    valid[ec, pos] = True
    for c in range(NC):
        xi_idx[c, ~valid[c]] = czero[c]
    pad_cnt = (~valid).sum(axis=1).astype(np.float32)
    assert np.all(~valid[:, -1]), "sentinel column must be padding"

    gzero = qzero[0]  # global slot with zero row
    xj_glob = np.where(valid, xj_idx, gzero).astype(np.int32)

    inv_cnt = (1.0 / np.maximum(cnt_of_slot.reshape(NC, SPC), 1.0)).astype(np.float32)

    g_of_slot = np.full(NSLOTS, -1, dtype=np.int64)
    real = node_of_slot < N_NODES
    g_of_slot[real] = batch[node_of_slot[real]]
    NGW = 8
    Bg = 0
    pools = [[None] * NGW for _ in range(NC)]
    for c in range(NC):
        gl = g_of_slot[c * SPC:(c + 1) * SPC]
        for gw in range(NGW):
            m = np.where((gl >= gw * 128) & (gl < (gw + 1) * 128))[0]
            pools[c][gw] = m
            Bg = max(Bg, (len(m) + 127) // 128)
    NPG = Bg * 128
    pool_idx = np.zeros((NC, NGW, NPG), dtype=np.int16)
    pool_gwl = np.full((NC, NGW, NPG), -1.0, dtype=np.float32)
    for c in range(NC):
        for gw in range(NGW):
            m = pools[c][gw]
            pool_idx[c, gw, :len(m)] = m.astype(np.int16)
            pool_idx[c, gw, len(m):] = czero[c]
            pool_gwl[c, gw, :len(m)] = (g_of_slot[c * SPC + m] - gw * 128).astype(np.float32)

    gcnt = np.bincount(batch, minlength=N_GRAPHS).astype(np.float32)
    inv_g = np.zeros(1024, dtype=np.float32)
    inv_g[:N_GRAPHS] = 1.0 / np.maximum(gcnt, 1.0)

    return dict(slot_of_node=slot_of_node, node_of_slot=node_of_slot,
                xj_glob=xj_glob, dstwin=dstwin, pad_cnt=pad_cnt,
                inv_cnt=inv_cnt, valid=valid, eorder=eorder, ec=ec, pos=pos,
                pool_idx=pool_idx, pool_gwl=pool_gwl, inv_g=inv_g, Bg=Bg)


def _bf(x):
    return np.ascontiguousarray(np.asarray(x, dtype=np.float32)).astype(ml_dtypes.bfloat16)


def _tile_w(w):
    K, M = w.shape
    nk, nm = (K + 127) // 128, (M + 127) // 128
    out = np.zeros((nk, nm, 128, 128), dtype=ml_dtypes.bfloat16)
    for i in range(nk):
        for j in range(nm):
            blk = np.asarray(w, dtype=np.float32)[i * 128:(i + 1) * 128, j * 128:(j + 1) * 128]
            out[i, j, :blk.shape[0], :blk.shape[1]] = _bf(blk)
    return out


# ============================ device kernel ============================

EQ = E_PAD // 4           # 12544 edges per msgT quarter
EQP = 12800               # quarter padded to 25x512
NSEGQ = EQP // 512        # 25


def _build(Bg, debug=False, phases=4):
    nc = bacc.Bacc("TRN2", target_bir_lowering=False, debug=False, num_devices=NC)

    def din(name, shape, dt):
        return nc.dram_tensor(name, shape, dt, kind="ExternalInput").ap()

    t_msgT = din("msgT", [128, EQP], BF16)
    t_xj = din("xj_idx", [128, E_PAD // 128], mybir.dt.int32)
    t_dstwin = din("dstwin", [128, E_PAD // 128], F32)
    t_dwrow = din("dwrow", [1, E_PAD], BF16)
    t_invcnt = din("invcnt", [128, NWIN], F32)
    t_padcnt = din("padcnt", [128, 1], F32)
    t_iota = din("iota", [128, 128], F32)
    t_iotap = din("iotap", [128, 1], F32)
    t_ident = din("ident", [128, 128], BF16)
    t_c1w = din("c1w", [3, 128, 128], BF16)
    t_c1b = din("c1b", [3, 128, 1], F32)
    t_c1gn = din("c1gn", [3, 3, 128, 1], F32)
    t_c2wa = din("c2wa", [2, 128, 128], BF16)
    t_c2wb = din("c2wb", [2, 128, 128], BF16)
    t_c2w2 = din("c2w2", [2, 2, 128, 128], BF16)
    t_c2b = din("c2b", [2, 2, 128, 1], F32)
    t_c2gn = din("c2gn", [2, 3, 2, 128, 1], F32)
    t_c3wa = din("c3wa", [2, 2, 128, 128], BF16)
    t_c3wbv = din("c3wbv", [2, 128, 256], BF16)
    t_c3b = din("c3b", [2, 128, 1], F32)
    t_c3gn = din("c3gn", [3, 2, 128, 1], F32)
    t_lw1 = din("lw1", [2, 2, 128, 128], BF16)
    t_lb1 = din("lb1", [2, 128, 1], F32)
    t_lw2 = din("lw2", [2, 128, 2], BF16)
    t_lb2 = din("lb2", [2, 1], F32)
    t_pidx = din("pool_idx", [8, 128, Bg], mybir.dt.int32)
    t_pgwl = din("pool_gwl", [128, 8 * Bg], F32)
    t_invg = din("invg", [128, 1], F32)

    o_out = nc.dram_tensor("out", [2, 128], F32, kind="ExternalOutput").ap()
    dbg = {}
    if debug:
        dbg["x1"] = nc.dram_tensor("dbg_x1", [NSLOTS, 128], BF16, kind="ExternalOutput").ap()
        dbg["x2"] = nc.dram_tensor("dbg_x2", [NSLOTS, 256], BF16, kind="ExternalOutput").ap()
        dbg["x3"] = nc.dram_tensor("dbg_x3", [SPC, 256], BF16, kind="ExternalOutput").ap()
        dbg["pool"] = nc.dram_tensor("dbg_pool", [128, 256], F32, kind="ExternalOutput").ap()

    with tile.TileContext(nc) as tc:
        with tc.tile_pool(name="dram", bufs=1, space="DRAM") as dram, \
             tc.tile_pool(name="cp", bufs=1) as cp, \
             tc.tile_pool(name="zp0", bufs=1) as zpool:
            # persistent SBUF z buffer (mb0 half of every conv's activations)
            zs = zpool.tile([128, E_PAD], BF16, name="zs")            # 12.8MB

            z_scr = [dram.tile([128, E_PAD], BF16, tag=f"zscr{i}", name=f"zscr{i}")
                     for i in range(2)]
            tab1_loc = dram.tile([SPC, 128], BF16)
            tab1 = dram.tile([NSLOTS, 128], BF16, addr_space="Shared")
            tab2_loc = dram.tile([SPC, 256], BF16)
            vt_loc = dram.tile([SPC, 256], BF16)
            vtab = dram.tile([NSLOTS, 256], BF16, addr_space="Shared")
            if debug:
                tab2 = dram.tile([NSLOTS, 256], BF16, addr_space="Shared")
            tab3_loc = dram.tile([SPC, 256], BF16)
            st_in = dram.tile([128, 8], F32)
            _agn = [0]
            pool_in = dram.tile([1024, 256], F32)
            pool_rs = dram.tile([128, 256], F32)

            ident = cp.tile([128, 128], BF16)
            nc.sync.dma_start(ident[:], t_ident[:])
            iota = cp.tile([128, 128], F32)
            nc.sync.dma_start(iota[:], t_iota[:])
            iotap = cp.tile([128, 1], F32)
            nc.sync.dma_start(iotap[:], t_iotap[:])
            invcnt = cp.tile([128, NWIN], F32)
            nc.sync.dma_start(invcnt[:], t_invcnt[:])
            dwin = cp.tile([128, E_PAD // 128], F32)
            nc.sync.dma_start(dwin[:], t_dstwin[:])
            padcnt = cp.tile([128, 1], F32)
            nc.sync.dma_start(padcnt[:], t_padcnt[:])
            zerocol = cp.tile([128, 1], F32)
            nc.vector.memset(zerocol[:], 0.0)
            iotab = cp.tile([128, 128], BF16)
            nc.vector.tensor_copy(iotab[:], iota[:])
            dwinb = cp.tile([128, E_PAD // 128], BF16)
            nc.vector.tensor_copy(dwinb[:], dwin[:])

            # ---------- helpers ----------
            def allgather_stats(s_acc, q_acc, n_mb, sb):
                # Shared DRAM allows a single writer inst: fresh tile per call
                _agn[0] += 1
                st_ag = dram.tile([128 * NC, 8], F32, addr_space="Shared",
                                  tag=f"st_ag{_agn[0]}", name=f"st_ag{_agn[0]}")
                st = sb.tile([128, 8], F32, tag="st_")
                nc.vector.memset(st[:], 0.0)
                nc.vector.tensor_copy(st[:, 0:n_mb], s_acc[:])
                nc.vector.tensor_copy(st[:, 4:4 + n_mb], q_acc[:])
                nc.sync.dma_start(st_in[:], st[:])
                nc.gpsimd.collective_compute(
                    "AllGather", AOP.bypass, replica_groups=[list(range(NC))],
                    ins=[st_in.opt()], outs=[st_ag.opt()])
                stg8 = sb.tile([128, 8, 8], F32, tag="stg8_")
                nc.sync.dma_start(stg8[:],
                                  st_ag[:].rearrange("(g p) j -> p g j", g=NC))
                stg = sb.tile([128, 8], F32, tag="stg_")
                nc.vector.tensor_reduce(
                    out=stg[:], in_=stg8[:].rearrange("p g j -> p j g"),
                    axis=AX.X, op=AOP.add)
                return stg

            def affine_from_stats(stg, n_mb, b_lin, gn, sb):
                A, Cc = [], []
                for mb in range(n_mb):
                    s = stg[:, mb:mb + 1]
                    q = stg[:, 4 + mb:5 + mb]
                    g, bgn, ms = gn[0][mb], gn[1][mb], gn[2][mb]
                    bl = b_lin[mb]
                    m = sb.tile([128, 1], F32, tag="af_m")
                    nc.vector.tensor_scalar(m[:], s, 1.0 / N_EDGES, None, AOP.mult)
                    nc.vector.tensor_tensor(m[:], m[:], bl, op=AOP.add)
                    e2 = sb.tile([128, 1], F32, tag="af_e2")
                    nc.vector.tensor_scalar(e2[:], q, 1.0 / N_EDGES, None, AOP.mult)
                    tmp = sb.tile([128, 1], F32, tag="af_t")
                    nc.vector.tensor_tensor(tmp[:], m[:], bl, op=AOP.mult)
                    nc.vector.tensor_scalar(tmp[:], tmp[:], 2.0, None, AOP.mult)
                    nc.vector.tensor_tensor(e2[:], e2[:], tmp[:], op=AOP.add)
                    nc.vector.tensor_tensor(tmp[:], bl, bl, op=AOP.mult)
                    nc.vector.tensor_tensor(e2[:], e2[:], tmp[:], op=AOP.subtract)
                    msm = sb.tile([128, 1], F32, tag="af_msm")
                    nc.vector.tensor_tensor(msm[:], ms, m[:], op=AOP.mult)
                    var = sb.tile([128, 1], F32, tag="af_v")
                    nc.vector.tensor_tensor(var[:], msm[:], msm[:], op=AOP.mult)
                    nc.vector.tensor_tensor(tmp[:], msm[:], m[:], op=AOP.mult)
                    nc.vector.tensor_scalar(tmp[:], tmp[:], 2.0, None, AOP.mult)
                    nc.vector.tensor_tensor(var[:], var[:], tmp[:], op=AOP.subtract)
                    nc.vector.tensor_tensor(var[:], var[:], e2[:], op=AOP.add)
                    a = sb.tile([128, 1], F32, tag="af_a")
                    nc.vector.tensor_scalar(var[:], var[:], EPS, None, AOP.add)
                    nc.scalar.activation(a[:], var[:], AFT.Sqrt)
                    nc.vector.reciprocal(a[:], a[:])
                    nc.vector.tensor_tensor(a[:], a[:], g, op=AOP.mult)
                    cc = sb.tile([128, 1], F32, tag="af_c")
                    nc.vector.tensor_tensor(cc[:], bl, msm[:], op=AOP.subtract)
                    nc.vector.tensor_tensor(cc[:], cc[:], a[:], op=AOP.mult)
                    nc.vector.tensor_tensor(cc[:], cc[:], bgn, op=AOP.add)
                    A.append(a)
                    Cc.append(cc)
                return A, Cc

            def zsink(zp_ap, dst_ap, s_col, sb, par=0):
                """PSUM -> bf16 dst copy fused with column-sum accumulation,
                s_col += colsum; alternates ACT / DVE by parity."""
                sa = sb.tile([128, 1], F32, tag="zk_sa")
                if par % 2 == 0:
                    nc.scalar.activation(dst_ap, zp_ap, AFT.Copy, accum_out=sa[:])
                else:
                    nc.vector.tensor_copy(dst_ap, zp_ap)
                    nc.vector.reduce_sum(out=sa[:], in_=zp_ap, axis=AX.X)
                nc.vector.tensor_tensor(s_col, s_col, sa[:], op=AOP.add)

            def blk_sq(src_ap, q_col, sb, par=0):
                """q_col += column sum-of-squares of bf16 [128, n] block;
                alternates ACT / DVE by parity."""
                n = src_ap.shape[-1]
                sq = sb.tile([128, BLK], BF16, tag="bs_sq")
                qa = sb.tile([128, 1], F32, tag="bs_qa")
                if par % 2 == 0:
                    nc.scalar.activation(sq[:, :n], src_ap, AFT.Square,
                                         accum_out=qa[:])
                else:
                    nc.vector.tensor_tensor(sq[:, :n], src_ap, src_ap,
                                            op=AOP.mult)
                    nc.vector.reduce_sum(out=qa[:], in_=sq[:, :n], axis=AX.X)
                nc.vector.tensor_tensor(q_col, q_col, qa[:], op=AOP.add)

            def sentinel_correct(s_acc, q_acc, zsent_cols, n_mb, sb):
                for mb in range(n_mb):
                    zs_ = zsent_cols[mb]
                    t1 = sb.tile([128, 1], F32, tag="sc_t1")
                    nc.vector.tensor_tensor(t1[:], zs_, padcnt[:], op=AOP.mult)
                    nc.vector.tensor_tensor(s_acc[:, mb:mb + 1], s_acc[:, mb:mb + 1],
                                            t1[:], op=AOP.subtract)
                    nc.vector.tensor_tensor(t1[:], zs_, zs_, op=AOP.mult)
                    nc.vector.tensor_tensor(t1[:], t1[:], padcnt[:], op=AOP.mult)
                    nc.vector.tensor_tensor(q_acc[:, mb:mb + 1], q_acc[:, mb:mb + 1],
                                            t1[:], op=AOP.subtract)

            def load_vec(t_ap, sb, tag):
                v = sb.tile([128, 1], F32, tag=tag)
                nc.sync.dma_start(v[:], t_ap)
                return v[:]

            def scatter_pass(zdram, n_mb, A, Cc, tab_loc, Cout, vw=None, vdst=None):
                """h = relu(A z + C) per mb; mb0 z from zs SBUF, mb1 from zdram.
                Mean-scatter into tab_loc DRAM; optionally also emit
                V = tab @ Wb rows into vdst."""
                with tc.tile_pool(name="sc_sb", bufs=2) as sb, \
                     tc.tile_pool(name="sc_tp", bufs=2, space="PSUM") as ps_tp, \
                     tc.tile_pool(name="sc_v", bufs=2, space="PSUM") as ps_v, \
                     tc.tile_pool(name="sc_sc", bufs=2, space="PSUM") as ps_sc:
                    for b in range(NBLK):
                        hs = []
                        for mb in range(n_mb):
                            if mb == 0:
                                zsrc_ap = zs[:, b * BLK:(b + 1) * BLK]
                            else:
                                zt = sb.tile([128, BLK], BF16, tag="sp_zt")
                                nc.sync.dma_start(zt[:], zdram[:, b * BLK:(b + 1) * BLK])
                                zsrc_ap = zt[:]
                            h = sb.tile([128, BLK], BF16, tag=f"sp_h{mb}")
                            nc.scalar.activation(h[:], zsrc_ap, AFT.Relu,
                                                 bias=Cc[mb], scale=A[mb])
                            hs.append(h)
                        hE = sb.tile([128, NCHUNK * Cout], BF16, tag="sp_hE")
                        for ch in range(NCHUNK):
                            for mb in range(n_mb):
                                tp = ps_tp.tile([128, 128], BF16, tag="sp_tp", space="PSUM")
                                nc.tensor.transpose(tp[:], hs[mb][:, ch * 128:(ch + 1) * 128],
                                                    ident[:])
                                dst = hE[:, ch * Cout + mb * 128:ch * Cout + (mb + 1) * 128]
                                if (ch + mb) % 2 == 0:
                                    nc.vector.tensor_copy(dst, tp[:])
                                else:
                                    nc.scalar.copy(dst, tp[:])
                        for w in range(NW_BLK):
                            gw = b * NW_BLK + w
                            sc = ps_sc.tile([128, Cout], F32, tag="sp_sc", space="PSUM")
                            for cb in range(B):
                                ch = w * B + cb
                                col = b * NCHUNK + ch
                                oh = sb.tile([128, 128], BF16, tag="sp_oh")
                                nc.vector.tensor_tensor(
                                    out=oh[:],
                                    in0=dwinb[:, col:col + 1].to_broadcast([128, 128]),
                                    in1=iotab[:], op=AOP.is_equal)
                                nc.tensor.matmul(sc[:], oh[:],
                                                 hE[:, ch * Cout:(ch + 1) * Cout],
                                                 start=(cb == 0), stop=(cb == B - 1))
                            nt = sb.tile([128, Cout], BF16, tag="sp_nt")
                            nc.vector.tensor_scalar(nt[:], sc[:], invcnt[:, gw:gw + 1],
                                                    None, AOP.mult)
                            nc.sync.dma_start(tab_loc[gw * WIN:(gw + 1) * WIN, :], nt[:])
                            if vw is not None:
                                # V = nt @ W3b for the next conv's src side
                                ntT = sb.tile([128, 256], BF16, tag="sp_ntT")
                                for kb in range(2):
                                    tpv = ps_tp.tile([128, 128], BF16, tag="sp_tp",
                                                     space="PSUM")
                                    nc.tensor.transpose(
                                        tpv[:], nt[:, kb * 128:(kb + 1) * 128],
                                        ident[:])
                                    if kb == 0:
                                        nc.vector.tensor_copy(ntT[:, 0:128], tpv[:])
                                    else:
                                        nc.scalar.copy(ntT[:, 128:256], tpv[:])
                                vps = ps_v.tile([128, 256], F32, tag="sp_v",
                                                space="PSUM")
                                for kb in range(2):
                                    nc.tensor.matmul(vps[:],
                                                     ntT[:, kb * 128:(kb + 1) * 128],
                                                     vw[kb][:],
                                                     start=(kb == 0), stop=(kb == 1))
                                nv = sb.tile([128, 256], BF16, tag="sp_nv")
                                nc.vector.tensor_copy(nv[:], vps[:])
                                nc.sync.dma_start(vdst[gw * WIN:(gw + 1) * WIN, :],
                                                  nv[:])

            # ======================= CONV 1 =======================
            with tc.tile_pool(name="c1sb", bufs=2) as sb:
                c1b = [[load_vec(t_c1b[i], sb, f"c1b{i}")] for i in range(3)]
                c1gn = [[[load_vec(t_c1gn[i, j], sb, f"c1gn{i}{j}")] for j in range(3)]
                        for i in range(3)]
                with tc.tile_pool(name="c1big", bufs=2) as bp, \
                     tc.tile_pool(name="c1ps", bufs=2, space="PSUM") as ps:
                    c1w = []
                    for i in range(3):
                        w = sb.tile([128, 128], BF16, tag=f"c1w{i}")
                        nc.sync.dma_start(w[:], t_c1w[i])
                        c1w.append(w)

                    # layer 1: z1 -> zs (4 partition-quarters of msgT)
                    s1 = sb.tile([128, 1], F32, tag="s1")
                    q1 = sb.tile([128, 1], F32, tag="q1")
                    nc.vector.memset(s1[:], 0.0)
                    nc.vector.memset(q1[:], 0.0)
                    with tc.tile_pool(name="c1msg", bufs=1) as msp:
                        msgT = msp.tile([128, EQP], BF16, name="msgT")
                        nc.sync.dma_start(msgT[:], t_msgT[:])
                        for q in range(4):
                            for g in range(7):          # groups of 4 segs
                                s0 = g * 4
                                nseg = min(4, NSEGQ - s0)
                                zp4 = ps.tile([128, 2048], F32, tag="zp4")
                                for si in range(nseg):
                                    s = s0 + si
                                    nc.tensor.matmul(
                                        zp4[:, si * 512:(si + 1) * 512],
                                        c1w[0][32 * q:32 * q + 10, :],
                                        msgT[32 * q:32 * q + 10,
                                             s * 512:(s + 1) * 512],
                                        start=True, stop=True,
                                        tile_position=(32 * q, 0) if q == 3 else None)
                                col = q * EQ + s0 * 512
                                nv = min(nseg * 512, EQ - s0 * 512)
                                zsink(zp4[:, :nv], zs[:, col:col + nv],
                                      s1[:, 0:1], sb, par=g)
                    for b in range(NBLK):
                        blk_sq(zs[:, b * BLK:(b + 1) * BLK], q1[:, 0:1], sb, par=b)
                    stg = allgather_stats(s1, q1, 1, sb)
                    A1, C1 = affine_from_stats(stg, 1, c1b[0], c1gn[0], sb)

                    # layers 2+3: z = W @ relu(aff(z_prev)), in-place in zs
                    ls_params = []
                    for li, wt in ((1, c1w[1]), (2, c1w[2])):
                        AA, CC = (A1, C1) if li == 1 else ls_params[0]
                        sL = sb.tile([128, 1], F32, tag=f"s{li + 1}")
                        qL = sb.tile([128, 1], F32, tag=f"q{li + 1}")
                        nc.vector.memset(sL[:], 0.0)
                        nc.vector.memset(qL[:], 0.0)
                        for b in range(NBLK):
                            h1 = bp.tile([128, BLK], BF16, tag="h1")
                            nc.scalar.activation(h1[:], zs[:, b * BLK:(b + 1) * BLK],
                                                 AFT.Relu, bias=CC[0], scale=AA[0])
                            for g, nseg in ((0, 4), (1, 3)):
                                zp4 = ps.tile([128, 2048], F32, tag="zp4")
                                for si in range(nseg):
                                    s = g * 4 + si
                                    nc.tensor.matmul(zp4[:, si * 512:(si + 1) * 512],
                                                     wt[:],
                                                     h1[:, s * 512:(s + 1) * 512],
                                                     start=True, stop=True)
                                col = b * BLK + g * 4 * 512
                                nv = nseg * 512
                                zsink(zp4[:, :nv], zs[:, col:col + nv],
                                      sL[:, 0:1], sb, par=g)
                            blk_sq(zs[:, b * BLK:(b + 1) * BLK], qL[:, 0:1], sb,
                                   par=b)
                        zsent = sb.tile([128, 1], F32, tag=f"zsent{li}")
                        nc.vector.tensor_copy(zsent[:], zs[:, E_PAD - 1:E_PAD])
                        sentinel_correct(sL, qL, [zsent[:]], 1, sb)
                        stgL = allgather_stats(sL, qL, 1, sb)
                        AL, CL = affine_from_stats(stgL, 1, c1b[li], c1gn[li], sb)
                        ls_params = [(AL, CL)]
                    A3, C3 = ls_params[0]

                scatter_pass(None, 1, A3, C3, tab1_loc, 128)

            nc.gpsimd.collective_compute(
                "AllGather", AOP.bypass, replica_groups=[list(range(NC))],
                ins=[tab1_loc.opt()], outs=[tab1.opt()])
            if debug:
                nc.sync.dma_start(dbg["x1"][:], tab1[:])

            # ============== gather-based first layer (conv2/conv3) ==============
            def gather_layer(tab_full, tab_loc_src, Cin, wa_t, wb_t, n_kb, zdram, sb):
                """z = WA @ xi + WB @ xj per 512-edge window;
                mo=0 -> zs SBUF, mo=1 -> zdram (block-staged)."""
                mb_in = Cin // 128
                s_acc = sb.tile([128, 2], F32, tag="gl_s")
                q_acc = sb.tile([128, 2], F32, tag="gl_q")
                nc.vector.memset(s_acc[:], 0.0)
                nc.vector.memset(q_acc[:], 0.0)
                ones1 = sb.tile([1, 128], BF16, tag="gl_ones")
                nc.vector.memset(ones1[:], 1.0)
                with tc.tile_pool(name="gl_g2", bufs=2) as g2, \
                     tc.tile_pool(name="gl_zw", bufs=2) as zwp, \
                     tc.tile_pool(name="gl_ps", bufs=2, space="PSUM") as ps, \
                     tc.tile_pool(name="gl_tp", bufs=2, space="PSUM") as ps_tp, \
                     tc.tile_pool(name="gl_xp", bufs=2, space="PSUM") as ps_xp:
                    was, wbs = [], []
                    for ki in range(n_kb):
                        for mo in range(2):
                            wta = sb.tile([128, 128], BF16, tag=f"gl_wa{ki}{mo}")
                            nc.sync.dma_start(wta[:], wa_t[ki, mo] if n_kb > 1 else wa_t[mo])
                            was.append(wta)
                            wtb = sb.tile([128, 128], BF16, tag=f"gl_wb{ki}{mo}")
                            nc.sync.dma_start(wtb[:], wb_t[ki, mo] if n_kb > 1 else wb_t[mo])
                            wbs.append(wtb)
                    for b in range(NBLK):
                        ixj = g2.tile([128, NCHUNK], mybir.dt.int32, tag="gl_ixj")
                        nc.sync.dma_start(ixj[:], t_xj[:, b * NCHUNK:(b + 1) * NCHUNK])
                        dwr = g2.tile([1, BLK], BF16, tag="gl_dwr")
                        nc.sync.dma_start(dwr[:], t_dwrow[0:1, b * BLK:(b + 1) * BLK])
                        twinb = g2.tile([128, NW_BLK * Cin], BF16, tag="gl_twinb")
                        nc.sync.dma_start(
                            twinb[:],
                            tab_loc_src[b * NW_BLK * WIN:(b + 1) * NW_BLK * WIN, :]
                            .rearrange("(w s) c -> s w c", w=NW_BLK))
                        zw = zwp.tile([128, BLK], BF16, tag="gl_zw")
                        for w in range(NW_BLK):
                            gw = b * NW_BLK + w
                            twin = twinb[:, w * Cin:(w + 1) * Cin]
                            # one-hot rows oh2[s, e] = (dstwin[e] == s) for the
                            # window's 512 edges, via K=1 broadcast matmul
                            bc = ps_xp.tile([128, 512], F32, tag="gl_bc", space="PSUM")
                            nc.tensor.matmul(bc[:], ones1[:],
                                             dwr[0:1, w * 512:(w + 1) * 512],
                                             start=True, stop=True)
                            oh2 = g2.tile([128, 512], BF16, tag="gl_oh2", bufs=4)
                            nc.vector.tensor_tensor(
                                out=oh2[:], in0=bc[:],
                                in1=iotap[:].to_broadcast([128, 512]),
                                op=AOP.is_equal)
                            # xi feature-major via twin.T @ oh2
                            xiT = g2.tile([128, mb_in * 512], BF16, tag="gl_xiT",
                                           bufs=4)
                            for kb in range(mb_in):
                                xp = ps_xp.tile([128, 512], F32, tag="gl_xp",
                                                space="PSUM")
                                nc.tensor.matmul(
                                    xp[:], twin[:, kb * 128:(kb + 1) * 128],
                                    oh2[:], start=True, stop=True)
                                if kb % 2 == 0:
                                    nc.scalar.copy(
                                        xiT[:, kb * 512:(kb + 1) * 512], xp[:])
                                else:
                                    nc.vector.tensor_copy(
                                        xiT[:, kb * 512:(kb + 1) * 512], xp[:])
                            # xj gather + transpose to feature-major
                            gxj = g2.tile([128, B * Cin], BF16, tag="gl_gxj",
                                          bufs=3)
                            for cb in range(B):
                                ch = w * B + cb
                                nc.gpsimd.indirect_dma_start(
                                    out=gxj[:, cb * Cin:(cb + 1) * Cin],
                                    out_offset=None,
                                    in_=tab_full[:],
                                    in_offset=bass.IndirectOffsetOnAxis(
                                        ap=ixj[:, ch:ch + 1], axis=0))
                            xjT = g2.tile([128, mb_in * 512], BF16, tag="gl_xjT",
                                           bufs=3)
                            for cb in range(B):
                                for kb in range(mb_in):
                                    tp2 = ps_tp.tile([128, 128], BF16, tag="gl_tp2",
                                                     space="PSUM")
                                    nc.tensor.transpose(
                                        tp2[:],
                                        gxj[:, cb * Cin + kb * 128:cb * Cin + (kb + 1) * 128],
                                        ident[:])
                                    nc.vector.tensor_copy(
                                        xjT[:, kb * 512 + cb * 128:kb * 512 + (cb + 1) * 128],
                                        tp2[:])
                            # z for this window's 512 edges
                            for mo in range(2):
                                zp = ps.tile([128, 512], F32, tag="gl_zp")
                                for ki in range(mb_in):
                                    nc.tensor.matmul(
                                        zp[:], was[ki * 2 + mo][:],
                                        xiT[:, ki * 512:(ki + 1) * 512],
                                        start=(ki == 0), stop=False)
                                for ki in range(mb_in):
                                    nc.tensor.matmul(
                                        zp[:], wbs[ki * 2 + mo][:],
                                        xjT[:, ki * 512:(ki + 1) * 512],
                                        start=False, stop=(ki == mb_in - 1))
                                if mo == 0:
                                    col = b * BLK + w * 512
                                    zsink(zp[:], zs[:, col:col + 512],
                                          s_acc[:, 0:1], sb, par=w + mo)
                                else:
                                    zsink(zp[:], zw[:, w * 512:(w + 1) * 512],
                                          s_acc[:, 1:2], sb, par=w + mo)
                        nc.sync.dma_start(zdram[:, b * BLK:(b + 1) * BLK], zw[:])
                        blk_sq(zs[:, b * BLK:(b + 1) * BLK], q_acc[:, 0:1], sb,
                               par=b)
                        blk_sq(zw[:], q_acc[:, 1:2], sb, par=b + 1)
                return s_acc, q_acc

            def gather_layer_v(vtab_full, tab_loc_src, wa_t, zmid, zout, sb):
                """conv3: z = WA @ xi + V[src]; two phases.
                Phase A computes the xi-side partial z (no dep on the V
                AllGather, so it overlaps it); phase B gathers V rows and
                transpose-accumulates them via identity matmuls."""
                Cin = 256
                mb_in = 2
                s_acc = sb.tile([128, 2], F32, tag="gv_s")
                q_acc = sb.tile([128, 2], F32, tag="gv_q")
                nc.vector.memset(s_acc[:], 0.0)
                nc.vector.memset(q_acc[:], 0.0)
                ones1 = sb.tile([1, 128], BF16, tag="gv_ones")
                nc.vector.memset(ones1[:], 1.0)
                was = []
                for ki in range(2):
                    for mo in range(2):
                        wta = sb.tile([128, 128], BF16, tag=f"gv_wa{ki}{mo}")
                        nc.sync.dma_start(wta[:], wa_t[ki, mo])
                        was.append(wta)
                # Phase A: xi partials -> zs (mo0) / zmid (mo1)
                with tc.tile_pool(name="ga_g2", bufs=2) as g2, \
                     tc.tile_pool(name="ga_zw", bufs=2) as zwp, \
                     tc.tile_pool(name="ga_ps", bufs=2, space="PSUM") as ps, \
                     tc.tile_pool(name="ga_xp", bufs=2, space="PSUM") as ps_xp:
                    for b in range(NBLK):
                        dwr = g2.tile([1, BLK], BF16, tag="ga_dwr")
                        nc.sync.dma_start(dwr[:], t_dwrow[0:1, b * BLK:(b + 1) * BLK])
                        zwa = zwp.tile([128, BLK], BF16, tag="ga_zwa")
                        twinb = g2.tile([128, NW_BLK * Cin], BF16, tag="ga_twinb")
                        nc.sync.dma_start(
                            twinb[:],
                            tab_loc_src[b * NW_BLK * WIN:(b + 1) * NW_BLK * WIN, :]
                            .rearrange("(w s) c -> s w c", w=NW_BLK))
                        for w in range(NW_BLK):
                            gw = b * NW_BLK + w
                            twin = twinb[:, w * Cin:(w + 1) * Cin]
                            bc = ps_xp.tile([128, 512], F32, tag="ga_bc", space="PSUM")
                            nc.tensor.matmul(bc[:], ones1[:],
                                             dwr[0:1, w * 512:(w + 1) * 512],
                                             start=True, stop=True)
                            oh2 = g2.tile([128, 512], BF16, tag="ga_oh2", bufs=4)
                            nc.vector.tensor_tensor(
                                out=oh2[:], in0=bc[:],
                                in1=iotap[:].to_broadcast([128, 512]),
                                op=AOP.is_equal)
                            xiT = g2.tile([128, mb_in * 512], BF16, tag="ga_xiT",
                                           bufs=4)
                            for kb in range(mb_in):
                                xp = ps_xp.tile([128, 512], F32, tag="ga_xp",
                                                space="PSUM")
                                nc.tensor.matmul(
                                    xp[:], twin[:, kb * 128:(kb + 1) * 128],
                                    oh2[:], start=True, stop=True)
                                if kb % 2 == 0:
                                    nc.vector.tensor_copy(
                                        xiT[:, kb * 512:(kb + 1) * 512], xp[:])
                                else:
                                    nc.scalar.copy(
                                        xiT[:, kb * 512:(kb + 1) * 512], xp[:])
                            for mo in range(2):
                                zp = ps.tile([128, 512], F32, tag="ga_zp")
                                for ki in range(mb_in):
                                    nc.tensor.matmul(
                                        zp[:], was[ki * 2 + mo][:],
                                        xiT[:, ki * 512:(ki + 1) * 512],
                                        start=(ki == 0), stop=(ki == mb_in - 1))
                                if mo == 0:
                                    col = b * BLK + w * 512
                                    nc.scalar.copy(zs[:, col:col + 512], zp[:])
                                else:
                                    nc.vector.tensor_copy(
                                        zwa[:, w * 512:(w + 1) * 512], zp[:])
                        nc.sync.dma_start(zmid[:, b * BLK:(b + 1) * BLK], zwa[:])
                # Phase B: V rows, transpose-accumulate via identity matmul
                with tc.tile_pool(name="gb_g2", bufs=2) as g2, \
                     tc.tile_pool(name="gb_zw", bufs=2) as zwp, \
                     tc.tile_pool(name="gb_ps", bufs=2, space="PSUM") as ps:
                    for b in range(NBLK):
                        ixj = g2.tile([128, NCHUNK], mybir.dt.int32, tag="gb_ixj")
                        nc.sync.dma_start(ixj[:], t_xj[:, b * NCHUNK:(b + 1) * NCHUNK])
                        zxw = zwp.tile([128, BLK], BF16, tag="gb_zxw")
                        nc.sync.dma_start(zxw[:], zmid[:, b * BLK:(b + 1) * BLK])
                        zw = zwp.tile([128, BLK], BF16, tag="gb_zw")
                        for w in range(NW_BLK):
                            gv = g2.tile([128, B * 256], BF16, tag="gb_gv", bufs=3)
                            for cb in range(B):
                                ch = w * B + cb
                                nc.gpsimd.indirect_dma_start(
                                    out=gv[:, cb * 256:(cb + 1) * 256],
                                    out_offset=None,
                                    in_=vtab_full[:],
                                    in_offset=bass.IndirectOffsetOnAxis(
                                        ap=ixj[:, ch:ch + 1], axis=0))
                            for mo in range(2):
                                zp = ps.tile([128, 512], F32, tag="gb_zp")
                                for cb in range(B):
                                    nc.tensor.matmul(
                                        zp[:, cb * 128:(cb + 1) * 128],
                                        gv[:, cb * 256 + mo * 128:
                                           cb * 256 + (mo + 1) * 128],
                                        ident[:], start=True, stop=True)
                                sa = sb.tile([128, 1], F32, tag="gb_sa")
                                col = b * BLK + w * 512
                                if mo == 0:
                                    nc.vector.tensor_tensor(
                                        zs[:, col:col + 512],
                                        zs[:, col:col + 512], zp[:], op=AOP.add)
                                    nc.vector.reduce_sum(
                                        out=sa[:], in_=zs[:, col:col + 512],
                                        axis=AX.X)
                                else:
                                    nc.vector.tensor_tensor(
                                        zw[:, w * 512:(w + 1) * 512],
                                        zxw[:, w * 512:(w + 1) * 512], zp[:],
                                        op=AOP.add)
                                    nc.vector.reduce_sum(
                                        out=sa[:], in_=zw[:, w * 512:(w + 1) * 512],
                                        axis=AX.X)
                                nc.vector.tensor_tensor(s_acc[:, mo:mo + 1],
                                                        s_acc[:, mo:mo + 1],
                                                        sa[:], op=AOP.add)
                        nc.sync.dma_start(zout[:, b * BLK:(b + 1) * BLK], zw[:])
                        blk_sq(zs[:, b * BLK:(b + 1) * BLK], q_acc[:, 0:1], sb,
                               par=b)
                        blk_sq(zw[:], q_acc[:, 1:2], sb, par=b + 1)
                return s_acc, q_acc

            # ======================= CONV 2 =======================
            if phases >= 2:
              with tc.tile_pool(name="c2sb", bufs=2) as sb:
                  c2b = [[load_vec(t_c2b[i, mb], sb, f"c2b{i}{mb}") for mb in range(2)]
                         for i in range(2)]
                  c2gn = [[[load_vec(t_c2gn[i, j, mb], sb, f"c2gn{i}{j}{mb}")
                            for mb in range(2)] for j in range(3)] for i in range(2)]
                  sA, qA = gather_layer(tab1, tab1_loc, 128, t_c2wa, t_c2wb, 1,
                                        z_scr[0], sb)
                  stg = allgather_stats(sA, qA, 2, sb)
                  A1, C1 = affine_from_stats(stg, 2, c2b[0], c2gn[0], sb)

                  # layer 2: z2 = W2 @ relu(aff(zA)); mb0 in-place zs, mb1 -> z_scr[1]
                  s2 = sb.tile([128, 2], F32, tag="c2s2")
                  q2 = sb.tile([128, 2], F32, tag="c2q2")
                  nc.vector.memset(s2[:], 0.0)
                  nc.vector.memset(q2[:], 0.0)
                  zsent = []
                  with tc.tile_pool(name="c2mid", bufs=2) as mp, \
                       tc.tile_pool(name="c2ps", bufs=2, space="PSUM") as ps:
                      w2s = []
                      for ki in range(2):
                          for mo in range(2):
                              w = sb.tile([128, 128], BF16, tag=f"c2w2{ki}{mo}")
                              nc.sync.dma_start(w[:], t_c2w2[ki, mo])
                              w2s.append(w)
                      for b in range(NBLK):
                          h1 = []
                          z1d = mp.tile([128, BLK], BF16, tag="c2z1r")
                          nc.sync.dma_start(z1d[:], z_scr[0][:, b * BLK:(b + 1) * BLK])
                          for mb in range(2):
                              zsrc_ap = (zs[:, b * BLK:(b + 1) * BLK] if mb == 0
                                         else z1d[:])
                              hh = mp.tile([128, BLK], BF16, tag=f"c2h1{mb}")
                              nc.scalar.activation(hh[:], zsrc_ap, AFT.Relu,
                                                   bias=C1[mb], scale=A1[mb])
                              h1.append(hh)
                          zw = mp.tile([128, BLK], BF16, tag="c2zw")
                          for mo in range(2):
                              for g, nseg in ((0, 4), (1, 3)):
                                  zp4 = ps.tile([128, 2048], F32, tag="c2zp4")
                                  for si in range(nseg):
                                      s = g * 4 + si
                                      for ki in range(2):
                                          nc.tensor.matmul(
                                              zp4[:, si * 512:(si + 1) * 512],
                                              w2s[ki * 2 + mo][:],
                                              h1[ki][:, s * 512:(s + 1) * 512],
                                              start=(ki == 0), stop=(ki == 1))
                                  col = g * 4 * 512
                                  nv = nseg * 512
                                  if mo == 0:
                                      zsink(zp4[:, :nv],
                                            zs[:, b * BLK + col:b * BLK + col + nv],
                                            s2[:, 0:1], sb, par=g + mo)
                                  else:
                                      zsink(zp4[:, :nv], zw[:, col:col + nv],
                                            s2[:, 1:2], sb, par=g + mo)
                          nc.sync.dma_start(z_scr[1][:, b * BLK:(b + 1) * BLK], zw[:])
                          blk_sq(zs[:, b * BLK:(b + 1) * BLK], q2[:, 0:1], sb, par=b)
                          blk_sq(zw[:], q2[:, 1:2], sb, par=b + 1)
                          if b == NBLK - 1:
                              zc0 = sb.tile([128, 1], F32, tag="c2zs0")
                              nc.vector.tensor_copy(zc0[:], zs[:, E_PAD - 1:E_PAD])
                              zc1 = sb.tile([128, 1], F32, tag="c2zs1")
                              nc.vector.tensor_copy(zc1[:], zw[:, BLK - 1:BLK])
                              zsent = [zc0[:], zc1[:]]
                  sentinel_correct(s2, q2, zsent, 2, sb)
                  stg2 = allgather_stats(s2, q2, 2, sb)
                  A2, C2 = affine_from_stats(stg2, 2, c2b[1], c2gn[1], sb)
                  vwt = []
                  for kb in range(2):
                      w = sb.tile([128, 256], BF16, tag=f"c2vw{kb}")
                      nc.sync.dma_start(w[:], t_c3wbv[kb])
                      vwt.append(w)
                  scatter_pass(z_scr[1], 2, A2, C2, tab2_loc, 256,
                               vw=vwt, vdst=vt_loc)

            nc.gpsimd.collective_compute(
                "AllGather", AOP.bypass, replica_groups=[list(range(NC))],
                ins=[vt_loc.opt()], outs=[vtab.opt()])
            if debug:
                nc.gpsimd.collective_compute(
                    "AllGather", AOP.bypass, replica_groups=[list(range(NC))],
                    ins=[tab2_loc.opt()], outs=[tab2.opt()])
                nc.sync.dma_start(dbg["x2"][:], tab2[:])

            # ======================= CONV 3 =======================
            if phases >= 3:
              with tc.tile_pool(name="c3sb", bufs=2) as sb:
                  c3b = [load_vec(t_c3b[mb], sb, f"c3b{mb}") for mb in range(2)]
                  c3gn = [[load_vec(t_c3gn[j, mb], sb, f"c3gn{j}{mb}") for mb in range(2)]
                          for j in range(3)]
                  sA, qA = gather_layer_v(vtab, tab2_loc, t_c3wa,
                                          z_scr[1], z_scr[0], sb)
                  stg = allgather_stats(sA, qA, 2, sb)
                  A1, C1 = affine_from_stats(stg, 2, c3b, c3gn, sb)
                  scatter_pass(z_scr[0], 2, A1, C1, tab3_loc, 256)

            if debug:
                nc.sync.dma_start(dbg["x3"][:], tab3_loc[:])

            # ======================= POOL + HEAD =======================
            if phases >= 4:
              with tc.tile_pool(name="p_sb", bufs=2) as sb, \
                 tc.tile_pool(name="p_ps", bufs=2, space="PSUM") as ps:
                  pgwl = sb.tile([128, 8 * Bg], F32, tag="p_pgwl")
                  nc.sync.dma_start(pgwl[:], t_pgwl[:])
                  for gw in range(8):
                      pidx = sb.tile([128, Bg], mybir.dt.int32, tag="p_idx")
                      nc.sync.dma_start(pidx[:], t_pidx[gw])
                      gp = sb.tile([128, Bg * 256], BF16, tag="p_gp")
                      for c in range(Bg):
                          nc.gpsimd.indirect_dma_start(
                              out=gp[:, c * 256:(c + 1) * 256], out_offset=None,
                              in_=tab3_loc[:],
                              in_offset=bass.IndirectOffsetOnAxis(
                                  ap=pidx[:, c:c + 1], axis=0))
                      pp = ps.tile([128, 256], F32, tag="p_pp", space="PSUM")
                      for c in range(Bg):
                          oh = sb.tile([128, 128], BF16, tag="p_oh")
                          nc.vector.tensor_tensor(
                              out=oh[:],
                              in0=pgwl[:, gw * Bg + c:gw * Bg + c + 1].to_broadcast([128, 128]),
                              in1=iota[:], op=AOP.is_equal)
                          nc.tensor.matmul(pp[:], oh[:], gp[:, c * 256:(c + 1) * 256],
                                           start=(c == 0), stop=(c == Bg - 1))
                      pf = sb.tile([128, 256], F32, tag="p_pf")
                      nc.vector.tensor_copy(pf[:], pp[:])
                      nc.sync.dma_start(pool_in[gw * 128:(gw + 1) * 128, :], pf[:])
                  nc.gpsimd.collective_compute(
                      "ReduceScatter", AOP.add, replica_groups=[list(range(NC))],
                      ins=[pool_in.opt()], outs=[pool_rs.opt()])
                  if debug:
                      nc.sync.dma_start(dbg["pool"][:], pool_rs[:])

                  invg = sb.tile([128, 1], F32, tag="p_invg")
                  nc.sync.dma_start(invg[:], t_invg[:])
                  lw1 = []
                  for ki in range(2):
                      for mo in range(2):
                          w = sb.tile([128, 128], BF16, tag=f"p_lw1{ki}{mo}")
                          nc.sync.dma_start(w[:], t_lw1[ki, mo])
                          lw1.append(w)
                  lw2 = []
                  for ki in range(2):
                      w = sb.tile([128, 2], BF16, tag=f"p_lw2{ki}")
                      nc.sync.dma_start(w[:], t_lw2[ki])
                      lw2.append(w)
                  lb1 = [load_vec(t_lb1[mb], sb, f"p_lb1{mb}") for mb in range(2)]
                  lb2 = sb.tile([2, 1], F32, tag="p_lb2")
                  nc.sync.dma_start(lb2[:], t_lb2[:])
                  g = sb.tile([128, 256], F32, tag="p_g")
                  nc.sync.dma_start(g[:], pool_rs[:])
                  gm = sb.tile([128, 256], BF16, tag="p_gm")
                  nc.vector.tensor_scalar(gm[:], g[:], invg[:, 0:1], None, AOP.mult)
                  gT = sb.tile([128, 2 * 128], BF16, tag="p_gT")
                  for kb in range(2):
                      tp = ps.tile([128, 128], BF16, tag="p_tp", space="PSUM")
                      nc.tensor.transpose(tp[:], gm[:, kb * 128:(kb + 1) * 128], ident[:])
                      nc.vector.tensor_copy(gT[:, kb * 128:(kb + 1) * 128], tp[:])
                  hT = sb.tile([128, 2 * 128], BF16, tag="p_hT")
                  for mo in range(2):
                      hp = ps.tile([128, 128], F32, tag="p_hp", space="PSUM")
                      for ki in range(2):
                          nc.tensor.matmul(hp[:], lw1[ki * 2 + mo][:],
                                           gT[:, ki * 128:(ki + 1) * 128],
                                           start=(ki == 0), stop=(ki == 1))
                      nc.scalar.activation(hT[:, mo * 128:(mo + 1) * 128], hp[:],
                                           AFT.Relu, bias=lb1[mo])
                  op_ = ps.tile([2, 128], F32, tag="p_op", space="PSUM")
                  for ki in range(2):
                      nc.tensor.matmul(op_[:], lw2[ki][:],
                                       hT[:, ki * 128:(ki + 1) * 128],
                                       start=(ki == 0), stop=(ki == 1))
                  ofin = sb.tile([2, 128], F32, tag="p_out")
                  nc.vector.tensor_scalar(ofin[:], op_[:], lb2[:], None, AOP.add)
                  nc.sync.dma_start(o_out[:], ofin[:])

    nc.compile()
    return nc


# ============================ entry point ============================


def kernel(**inputs):
    x = np.asarray(inputs["x"], dtype=np.float32)
    edge_index = np.asarray(inputs["edge_index"])
    batch = np.asarray(inputs["batch"])

    meta = _pack(edge_index, batch)
    Bg = meta["Bg"]

    import os as _os
    phases = int(_os.environ.get("KPHASES", "4"))
    key = ("mod", Bg, phases, _DEBUG[0])
    if key not in _cache:
        _cache[key] = _build(Bg, debug=bool(inputs.get("_debug", False)) or _DEBUG[0],
                             phases=phases)
    nc = _cache[key]

    # ---- per-core input arrays ----
    src = np.asarray(edge_index[0], dtype=np.int64)
    dst = np.asarray(edge_index[1], dtype=np.int64)

    # conv1 msgT: [core, 128, EQP] bf16; quarter q of the edge range lives on
    # partition rows 32q..32q+10, cols 0..EQ (padded to EQP with zeros)
    xi_v = x[dst]
    xj_v = x[src]
    msg = np.concatenate([xi_v, xj_v - xi_v], axis=1)       # [E, 10]
    msg_full = np.zeros((NC, E_PAD, 10), dtype=np.float32)
    ec, pos = meta["ec"], meta["pos"]
    msg_full[ec, pos] = msg[meta["eorder"]]
    msgT = np.zeros((NC, 128, EQP), dtype=ml_dtypes.bfloat16)
    for q in range(4):
        msgT[:, 32 * q:32 * q + 10, :EQ] = _bf(
            msg_full[:, q * EQ:(q + 1) * EQ].transpose(0, 2, 1))

    dstwin = meta["dstwin"]  # [NC, E_PAD]
    dwin_in = np.ascontiguousarray(
        dstwin.reshape(NC, E_PAD // 128, 128).transpose(0, 2, 1)).astype(np.float32)
    invcnt_in = np.ascontiguousarray(
        meta["inv_cnt"].reshape(NC, NWIN, 128).transpose(0, 2, 1)).astype(np.float32)
    padcnt_in = np.repeat(meta["pad_cnt"][:, None], 128, axis=1)[:, :, None].astype(np.float32)

    iota_in = np.broadcast_to(np.arange(128, dtype=np.float32)[None, :], (128, 128))
    iota_in = np.ascontiguousarray(iota_in)
    iotap_in = np.arange(128, dtype=np.float32).reshape(128, 1)
    ident_in = np.eye(128, dtype=np.float32).astype(ml_dtypes.bfloat16)
    dwrow_in = dstwin.astype(ml_dtypes.bfloat16).reshape(NC, 1, E_PAD)

    xj_in = np.ascontiguousarray(
        meta["xj_glob"].reshape(NC, E_PAD // 128, 128).transpose(0, 2, 1)).astype(np.int32)

    # weights
    c1w = np.zeros((3, 128, 128), dtype=ml_dtypes.bfloat16)
    for q in range(4):
        c1w[0, 32 * q:32 * q + 10, :] = _bf(inputs["c1_w1"])
    c1w[1] = _bf(inputs["c1_w2"])
    c1w[2] = _bf(inputs["c1_w3"])
    c1b = np.stack([np.asarray(inputs[f"c1_b{i}"], dtype=np.float32).reshape(128, 1)
                    for i in (1, 2, 3)])
    c1gn = np.stack([np.asarray(inputs[f"c1_gn{i}"], dtype=np.float32).reshape(3, 128, 1)
                     for i in (1, 2, 3)])

    w2a = np.asarray(inputs["c2_w1"], dtype=np.float32)   # [256, 256]
    WA2 = w2a[:128] - w2a[128:]
    WB2 = w2a[128:]
    c2wa = _tile_w(WA2)[0]                                # [2, 128, 128]
    c2wb = _tile_w(WB2)[0]
    c2w2 = _tile_w(np.asarray(inputs["c2_w2"], dtype=np.float32))  # [2,2,128,128]
    c2b = np.stack([np.asarray(inputs["c2_b1"], dtype=np.float32).reshape(2, 128, 1),
                    np.asarray(inputs["c2_b2"], dtype=np.float32).reshape(2, 128, 1)])
    c2gn = np.stack([np.asarray(inputs["c2_gn1"], dtype=np.float32).reshape(3, 2, 128, 1),
                     np.asarray(inputs["c2_gn2"], dtype=np.float32).reshape(3, 2, 128, 1)])

    w3a = np.asarray(inputs["c3_w1"], dtype=np.float32)   # [512, 256]
    WA3 = w3a[:256] - w3a[256:]
    WB3 = w3a[256:]
    c3wa = _tile_w(WA3)                                   # [2,2,128,128]
    c3wbv = np.stack([_bf(WB3[:128]), _bf(WB3[128:])])    # [2,128,256]
    c3b = np.asarray(inputs["c3_b1"], dtype=np.float32).reshape(2, 128, 1)
    c3gn = np.asarray(inputs["c3_gn1"], dtype=np.float32).reshape(3, 2, 128, 1)

    lw1 = _tile_w(np.asarray(inputs["lin_w1"], dtype=np.float32))
    lb1 = np.asarray(inputs["lin_b1"], dtype=np.float32).reshape(2, 128, 1)
    lw2_f = np.asarray(inputs["lin_w2"], dtype=np.float32)  # [256, 2]
    lw2 = np.stack([_bf(lw2_f[:128]), _bf(lw2_f[128:])])    # [2, 128, 2]
    lb2 = np.asarray(inputs["lin_b2"], dtype=np.float32).reshape(2, 1)

    Bg0 = meta["Bg"]
    pidx_in = np.ascontiguousarray(
        meta["pool_idx"].astype(np.int32).reshape(NC, 8, Bg0, 128).transpose(0, 1, 3, 2))
    pgwl = meta["pool_gwl"]                # [NC, 8, NPG]
    Bg_ = meta["Bg"]
    pgwl_in = np.ascontiguousarray(
        pgwl.reshape(NC, 8, Bg_, 128).transpose(0, 3, 1, 2)).reshape(NC, 128, 8 * Bg_)
    # per-core shard of 1/graph-count (graphs c*128 + p)
    invg_in = meta["inv_g"].reshape(8, 128)[:, :, None].astype(np.float32)

    in_maps = []
    for c in range(NC):
        im = {
            "msgT": msgT[c],
            "xj_idx": xj_in[c],
            "dstwin": dwin_in[c],
            "dwrow": np.ascontiguousarray(dwrow_in[c]),
            "invcnt": invcnt_in[c],
            "padcnt": padcnt_in[c],
            "iota": iota_in,
            "iotap": iotap_in,
            "ident": ident_in,
            "c1w": c1w, "c1b": c1b, "c1gn": c1gn,
            "c2wa": c2wa, "c2wb": c2wb, "c2w2": c2w2, "c2b": c2b, "c2gn": c2gn,
            "c3wa": c3wa, "c3wbv": c3wbv, "c3b": c3b, "c3gn": c3gn,
            "lw1": lw1, "lb1": lb1, "lw2": lw2, "lb2": lb2,
            "pool_idx": pidx_in[c],
            "pool_gwl": pgwl_in[c].astype(np.float32),
            "invg": np.ascontiguousarray(invg_in[c]),
        }
        in_maps.append(im)

    res = run_bass_kernel_spmd(nc, in_maps, core_ids=list(range(NC)),
                               trace=_TRACE[0])
    kernel.last_result = res
    kernel.last_meta = meta
    out = np.concatenate([res.results[c]["out"] for c in range(NC)], axis=1)
    return np.ascontiguousarray(out[:, :N_GRAPHS].T).astype(np.float32)


_DEBUG = [False]
_TRACE = [False]
